# revision 13
# baseline (speedup 1.0000x reference)
"""Trainium2 Bass kernel for nn_Decoder_88605175316972.

Sharding: data-parallel over batch (4 samples) x tensor-parallel over heads
(2 groups of 4 heads) = 8 cores. Core c handles sample c//2, head-group c%2.
lm_W is column-sharded over vocab (each core computes 16384 padded columns).
Pairwise on-device AllReduce (replica groups [0,1],[2,3],[4,5],[6,7]) after
each attention output projection and each FFN second matmul. Logits and
layer-0 cross-attention weights are gathered on the host.

All activations are kept feature-major [feature(partitions), token(free)] so
layernorm/bias broadcasts are per-partition ops; softmax is computed on
transposed attention scores with row-sums done by ones-vector matmuls.
Matmuls run as float32r (full-rate fp32 on the PE).
"""

import numpy as np

import concourse.bass as bass
import concourse.mybir as mybir
import concourse.tile as tile
from concourse import bacc
from concourse.bass_utils import run_bass_kernel_spmd

V = 32000
D = 512
L = 2
H = 8
B = 4
TS = 512
TT = 512

NP = 128          # partitions
NT = D // NP      # 4 tiles per 512-dim
HH = H // 2       # heads per core
HID = 1024        # ffn hidden shard (4*D/2)
VS = 16384        # vocab shard per core (padded vocab 32768)
NVT = VS // 512   # vocab col-tiles per core
RG = [[0, 1], [2, 3], [4, 5], [6, 7]]
SCALE = 1.0 / float(np.sqrt(np.float32(D)))

F32 = mybir.dt.float32
F32R = mybir.dt.float32r
AF = mybir.ActivationFunctionType
ALU = mybir.AluOpType


def _r(ap):
    """[512, N] dram view -> [128, 4, N] partition-tiled view."""
    return ap.rearrange("(c p) n -> p c n", p=NP)


def build_nc(use_mask2: bool, ln_affine: bool):
    nc = bacc.Bacc("TRN2", target_bir_lowering=False, debug=False, num_devices=8)
    d = {}

    def inp(name, shape, dt=F32R):
        d[name] = nc.dram_tensor(name, list(shape), dt, kind="ExternalInput")

    inp("x0T", [D, TT])
    inp("esT", [D, TS])
    inp("maskT", [TT, TT])
    if use_mask2:
        inp("mask2T", [TS, TT])
    for p in ("1", "2"):
        inp("wq" + p, [L, HH, D, D])
        inp("wk" + p, [L, HH, D, D])
        inp("wv" + p, [L, HH, D, D])
        inp("bq" + p, [L, HH, D], F32)
        inp("bk" + p, [L, HH, D], F32)
        inp("wo" + p, [L, HH * D, D])
        inp("c" + p, [L, D], F32)
    inp("wff1", [L, D, HID])
    inp("bff1", [L, HID], F32)
    inp("wff2", [L, HID, D])
    inp("cff", [L, D], F32)
    if ln_affine:
        inp("lng", [L, 3, D], F32)
        inp("lnb", [L, 3, D], F32)
    inp("lmw", [D, VS])

    logits_d = nc.dram_tensor("logits", [TT, VS], F32, kind="ExternalOutput")
    att0_d = nc.dram_tensor("att0", [HH, TS, TT], F32, kind="ExternalOutput")

    def mm(ps, lhsT, rhs, start, stop):
        nc.tensor.matmul(ps, lhsT, rhs, start=start, stop=stop)

    with tile.TileContext(nc) as tc, \
         nc.allow_low_precision(reason="float32r matmul pipeline (tf32-class)"):
        with tc.tile_pool(name="const", bufs=1) as cpool, \
             tc.tile_pool(name="xp", bufs=2) as xpool, \
             tc.tile_pool(name="act", bufs=1) as apool, \
             tc.tile_pool(name="wp", bufs=1) as wpool, \
             tc.tile_pool(name="lmp", bufs=2) as lmpool, \
             tc.tile_pool(name="row", bufs=1) as rpool, \
             tc.tile_pool(name="psw", bufs=2, space="PSUM") as psw, \
             tc.tile_pool(name="psa", bufs=1, space="PSUM") as psa, \
             tc.tile_pool(name="psr", bufs=1, space="PSUM") as psr, \
             tc.tile_pool(name="dramp", bufs=2, space="DRAM") as dpool:

            ones_col = cpool.tile([NP, 1], F32R, name="ones_col")
            ones_row = cpool.tile([1, NP], F32R, name="ones_row")
            ones_f32 = cpool.tile([NP, 1], F32, name="ones_f32")
            eps_sb = cpool.tile([1, 1], F32, name="eps_sb")
            nc.vector.memset(ones_f32[:], 1.0)
            nc.vector.memset(eps_sb[:], 1e-5)
            nc.vector.tensor_copy(ones_col[:], ones_f32[:])
            nc.vector.tensor_copy(ones_row[:], ones_f32[:1, :1].to_broadcast([1, NP]))

            mask_sb = cpool.tile([NP, NT, TT], F32R, name="mask_sb")
            nc.sync.dma_start(mask_sb[:], _r(d["maskT"][:]))
            if use_mask2:
                mask2_sb = cpool.tile([NP, NT, TT], F32R, name="mask2_sb")
                nc.sync.dma_start(mask2_sb[:], _r(d["mask2T"][:]))
            es_sb = cpool.tile([NP, NT, TS], F32R, name="es_sb")
            nc.sync.dma_start(es_sb[:], _r(d["esT"][:]))

            # biases / small vectors, partition-tiled once
            bq_sb, bk_sb, c_sb = {}, {}, {}
            for p in ("1", "2"):
                bq_sb[p] = cpool.tile([NP, L, HH, NT], F32, name=f"bq{p}_sb")
                nc.sync.dma_start(
                    bq_sb[p][:], d["bq" + p][:].rearrange("l h (c p) -> p l h c", p=NP))
                bk_sb[p] = cpool.tile([NP, L, HH, NT], F32, name=f"bk{p}_sb")
                nc.sync.dma_start(
                    bk_sb[p][:], d["bk" + p][:].rearrange("l h (c p) -> p l h c", p=NP))
                c_sb[p] = cpool.tile([NP, L, NT], F32, name=f"c{p}_sb")
                nc.sync.dma_start(
                    c_sb[p][:], d["c" + p][:].rearrange("l (c p) -> p l c", p=NP))
            cff_sb = cpool.tile([NP, L, NT], F32, name="cff_sb")
            nc.sync.dma_start(cff_sb[:],
                              d["cff"][:].rearrange("l (c p) -> p l c", p=NP))
            bff1_sb = cpool.tile([NP, L, HID // NP], F32, name="bff1_sb")
            nc.sync.dma_start(bff1_sb[:],
                              d["bff1"][:].rearrange("l (c p) -> p l c", p=NP))
            if ln_affine:
                lng_sb = cpool.tile([NP, L, 3, NT], F32, name="lng_sb")
                nc.sync.dma_start(
                    lng_sb[:], d["lng"][:].rearrange("l k (c p) -> p l k c", p=NP))
                lnb_sb = cpool.tile([NP, L, 3, NT], F32, name="lnb_sb")
                nc.sync.dma_start(
                    lnb_sb[:], d["lnb"][:].rearrange("l k (c p) -> p l k c", p=NP))

            x_sb = xpool.tile([NP, NT, TT], F32R, name="x0", tag="x")
            nc.sync.dma_start(x_sb[:], _r(d["x0T"][:]))

            def all_reduce(src_sb, key):
                """src_sb [128, NT, 512] fm partial -> summed-over-pair tile."""
                ar_in = dpool.tile([D, TT], F32, name=f"arin_{key}", tag="ar_in")
                ar_out = dpool.tile([D, TT], F32, name=f"arout_{key}", tag="ar_out")
                nc.sync.dma_start(_r(ar_in), src_sb[:])
                nc.gpsimd.collective_compute(
                    "AllReduce", ALU.add, replica_groups=RG,
                    ins=[ar_in.opt()], outs=[ar_out.opt()])
                af = xpool.tile([NP, NT, TT], F32, name=f"af_{key}", tag="af", bufs=1)
                nc.sync.dma_start(af[:], _r(ar_out))
                return af

            def layernorm(resid, l, which, key):
                """resid [128, NT, 512] fm -> new x tile (normalized over feature dim)."""
                s1 = psr.tile([1, TT], F32, name=f"s1_{key}", tag="rs")
                sq = apool.tile([NP, NT, TT], F32R, name=f"sq_{key}", tag="sq")
                s2 = psr.tile([1, TT], F32, name=f"s2_{key}", tag="rs2")
                for c in range(NT):
                    mm(s1[:], ones_col[:], resid[:, c, :], c == 0, c == NT - 1)
                for c in range(NT):
                    nc.scalar.activation(sq[:, c, :], resid[:, c, :], AF.Square)
                for c in range(NT):
                    mm(s2[:], ones_col[:], sq[:, c, :], c == 0, c == NT - 1)
                mu = rpool.tile([1, TT], F32, name=f"mu_{key}", tag="mu")
                nc.vector.tensor_scalar_mul(mu[:], s1[:], 1.0 / D)
                # var = s2/D - mu^2, then sqrt(var+eps) in place
                var = rpool.tile([1, TT], F32, name=f"var_{key}", tag="var")
                nc.vector.tensor_mul(var[:], mu[:], mu[:])
                nc.vector.scalar_tensor_tensor(
                    var[:], s2[:], 1.0 / D, var[:], ALU.mult, ALU.subtract)
                nc.scalar.activation(var[:], var[:], AF.Sqrt, bias=eps_sb[:])
                rstd = rpool.tile([1, TT], F32R, name=f"rstd_{key}", tag="rstd")
                nc.vector.reciprocal(rstd[:], var[:])
                mr = rpool.tile([1, TT], F32R, name=f"mr_{key}", tag="mr")
                nc.vector.tensor_mul(mr[:], mu[:], rstd[:])
                rb_ps = psw.tile([NP, TT], F32, name=f"rbps_{key}", tag="w")
                mm(rb_ps[:], ones_row[:], rstd[:], True, True)
                rb = apool.tile([NP, TT], F32, name=f"rb_{key}", tag="rb")
                nc.scalar.copy(rb[:], rb_ps[:])
                mb_ps = psw.tile([NP, TT], F32, name=f"mbps_{key}", tag="w")
                mm(mb_ps[:], ones_row[:], mr[:], True, True)
                mb = apool.tile([NP, TT], F32, name=f"mb_{key}", tag="mb")
                nc.scalar.copy(mb[:], mb_ps[:])
                xn = xpool.tile([NP, NT, TT], F32R, name=f"x_{key}", tag="x")
                for c in range(NT):
                    nc.vector.tensor_mul(xn[:, c, :], resid[:, c, :], rb[:])
                    nc.vector.tensor_sub(xn[:, c, :], xn[:, c, :], mb[:])
                    if ln_affine:
                        nc.vector.tensor_scalar(
                            xn[:, c, :], xn[:, c, :], lng_sb[:, l, which, c:c + 1],
                            lnb_sb[:, l, which, c:c + 1], ALU.mult, ALU.add)
                return xn

            def residual_ln(x_old, af, cvec_ap, l, which, key):
                resid = apool.tile([NP, NT, TT], F32R, name=f"res_{key}", tag="res")
                for c in range(NT):
                    nc.vector.scalar_tensor_tensor(
                        resid[:, c, :], af[:, c, :], cvec_ap[c], x_old[:, c, :],
                        ALU.add, ALU.add)
                return resid

            def attention(x_cur, kv_sb, kv_len, p, l, msk, out_att0, key):
                """One masked MHA block (4 heads) -> all-reduced + LN'd new x."""
                a_ps = [psa.tile([NP, TT], F32, name=f"aps{dt}_{key}", tag=f"a{dt}")
                        for dt in range(NT)]
                NK = kv_len // NP
                for h in range(HH):
                    wq_t = wpool.tile([NP, NT, D], F32R, name=f"wq_{key}_{h}", tag="wq")
                    nc.sync.dma_start(wq_t[:], _r(d["wq" + p][l, h]))
                    wk_t = wpool.tile([NP, NT, D], F32R, name=f"wk_{key}_{h}", tag="wk")
                    nc.sync.dma_start(wk_t[:], _r(d["wk" + p][l, h]))
                    wv_t = wpool.tile([NP, NT, D], F32R, name=f"wv_{key}_{h}", tag="wv")
                    nc.sync.dma_start(wv_t[:], _r(d["wv" + p][l, h]))
                    wo_t = wpool.tile([NP, NT, D], F32R, name=f"wo_{key}_{h}", tag="wo")
                    nc.sync.dma_start(wo_t[:], _r(d["wo" + p][l, h * D:(h + 1) * D]))

                    # Q/K feature-major [E, Tq] with fused per-partition bias
                    q_sb = apool.tile([NP, NT, TT], F32R, name=f"q_{key}_{h}", tag="q")
                    for e in range(NT):
                        ps = psw.tile([NP, TT], F32, name=f"qps_{key}_{h}_{e}", tag="w")
                        for c in range(NT):
                            mm(ps[:], wq_t[:, c, e * NP:(e + 1) * NP],
                               x_cur[:, c, :], c == 0, c == NT - 1)
                        nc.scalar.activation(q_sb[:, e, :], ps[:], AF.Identity,
                                             bias=bq_sb[p][:, l, h, e:e + 1])
                    k_sb = apool.tile([NP, NK, kv_len], F32R, name=f"k_{key}_{h}", tag="k")
                    for e in range(NT):
                        ps = psw.tile([NP, kv_len], F32, name=f"kps_{key}_{h}_{e}", tag="w")
                        for c in range(NT):
                            mm(ps[:], wk_t[:, c, e * NP:(e + 1) * NP],
                               kv_sb[:, c, :], c == 0, c == NT - 1)
                        nc.scalar.activation(k_sb[:, e, :], ps[:], AF.Identity,
                                             bias=bk_sb[p][:, l, h, e:e + 1])
                    # V token-major [Tk, E]
                    v_sb = apool.tile([NP, NK, D], F32R, name=f"v_{key}_{h}", tag="v")
                    for t in range(NK):
                        ps = psw.tile([NP, D], F32, name=f"vps_{key}_{h}_{t}", tag="w")
                        for c in range(NT):
                            mm(ps[:], kv_sb[:, c, t * NP:(t + 1) * NP],
                               wv_t[:, c, :], c == 0, c == NT - 1)
                        nc.scalar.copy(v_sb[:, t, :], ps[:])
                    # PT = exp(scale * K^T Q) * mask   [Tk, Tq]
                    pt_sb = apool.tile([NP, NK, TT], F32R, name=f"pt_{key}_{h}", tag="pt")
                    for t in range(NK):
                        ps = psw.tile([NP, TT], F32, name=f"ptps_{key}_{h}_{t}", tag="w")
                        for c in range(NT):
                            mm(ps[:], k_sb[:, c, t * NP:(t + 1) * NP],
                               q_sb[:, c, :], c == 0, c == NT - 1)
                        nc.scalar.activation(pt_sb[:, t, :], ps[:], AF.Exp, scale=SCALE)
                        if msk is not None:
                            nc.vector.tensor_mul(pt_sb[:, t, :], pt_sb[:, t, :],
                                                 msk[:, t, :])
                    # softmax denominator and reciprocal broadcast
                    rs = psr.tile([1, TT], F32, name=f"rs_{key}_{h}", tag="rs")
                    for t in range(NK):
                        mm(rs[:], ones_col[:], pt_sb[:, t, :], t == 0, t == NK - 1)
                    rinv = rpool.tile([1, TT], F32R, name=f"rinv_{key}_{h}", tag="rinv")
                    nc.vector.reciprocal(rinv[:], rs[:])
                    bc_ps = psw.tile([NP, TT], F32, name=f"bcps_{key}_{h}", tag="w")
                    mm(bc_ps[:], ones_row[:], rinv[:], True, True)
                    bc = apool.tile([NP, TT], F32, name=f"bc_{key}_{h}", tag="bc")
                    nc.scalar.copy(bc[:], bc_ps[:])
                    # O^T = V^T P^T, normalized on evacuation  [E, Tq]
                    o_sb = apool.tile([NP, NT, TT], F32R, name=f"o_{key}_{h}", tag="o")
                    for e in range(NT):
                        ps = psw.tile([NP, TT], F32, name=f"ops_{key}_{h}_{e}", tag="w")
                        for t in range(NK):
                            mm(ps[:], v_sb[:, t, e * NP:(e + 1) * NP],
                               pt_sb[:, t, :], t == 0, t == NK - 1)
                        nc.vector.tensor_mul(o_sb[:, e, :], ps[:], bc[:])
                    if out_att0:
                        ptn = apool.tile([NP, NK, TT], F32, name=f"ptn_{key}_{h}", tag="sq")
                        for t in range(NK):
                            nc.vector.tensor_mul(ptn[:, t, :], pt_sb[:, t, :], bc[:])
                            nc.sync.dma_start(
                                att0_d[h, t * NP:(t + 1) * NP, :], ptn[:, t, :])
                    # accumulate Wo^T O^T into a_ps (feature-major [D, Tq])
                    for dt in range(NT):
                        for e in range(NT):
                            mm(a_ps[dt][:], wo_t[:, e, dt * NP:(dt + 1) * NP],
                               o_sb[:, e, :], h == 0 and e == 0,
                               h == HH - 1 and e == NT - 1)
                a_sb = apool.tile([NP, NT, TT], F32, name=f"a_{key}", tag="asb")
                for dt in range(NT):
                    nc.scalar.copy(a_sb[:, dt, :], a_ps[dt][:])
                af = all_reduce(a_sb, key)
                cvec = [c_sb[p][:, l, c:c + 1] for c in range(NT)]
                resid = residual_ln(x_cur, af, cvec, l, 0 if p == "1" else 1, key)
                return layernorm(resid, l, 0 if p == "1" else 1, key)

            for l in range(L):
                x_sb = attention(x_sb, x_sb, TT, "1", l, mask_sb, False, f"s{l}")
                x_sb = attention(x_sb, es_sb, TS, "2", l,
                                 mask2_sb if use_mask2 else None, l == 0, f"c{l}")
                # FFN: h1 = relu(W1^T x + b1) feature-major [HID, Tq]
                wf1_t = wpool.tile([NP, NT, HID], F32R, name=f"wf1_{l}", tag="wff")
                nc.sync.dma_start(wf1_t[:], _r(d["wff1"][l]))
                h1_sb = apool.tile([NP, HID // NP, TT], F32R, name=f"h1_{l}", tag="h1")
                for m in range(HID // NP):
                    ps = psw.tile([NP, TT], F32, name=f"h1ps_{l}_{m}", tag="w")
                    for c in range(NT):
                        mm(ps[:], wf1_t[:, c, m * NP:(m + 1) * NP],
                           x_sb[:, c, :], c == 0, c == NT - 1)
                    nc.scalar.activation(h1_sb[:, m, :], ps[:], AF.Relu,
                                         bias=bff1_sb[:, l, m:m + 1])
                wf2_t = wpool.tile([NP, HID // NP, D], F32R, name=f"wf2_{l}", tag="wff")
                nc.sync.dma_start(
                    wf2_t[:], d["wff2"][l].rearrange("(c p) n -> p c n", p=NP))
                f_ps = [psa.tile([NP, TT], F32, name=f"fps{dt}_{l}", tag=f"a{dt}")
                        for dt in range(NT)]
                for dt in range(NT):
                    for c in range(HID // NP):
                        mm(f_ps[dt][:], wf2_t[:, c, dt * NP:(dt + 1) * NP],
                           h1_sb[:, c, :], c == 0, c == HID // NP - 1)
                ff_sb = apool.tile([NP, NT, TT], F32, name=f"ff_{l}", tag="asb")
                for dt in range(NT):
                    nc.scalar.copy(ff_sb[:, dt, :], f_ps[dt][:])
                af = all_reduce(ff_sb, f"f{l}")
                cvec = [cff_sb[:, l, c:c + 1] for c in range(NT)]
                resid = residual_ln(x_sb, af, cvec, l, 2, f"f{l}")
                x_sb = layernorm(resid, l, 2, f"f{l}")

            # LM head: logits token-major [Tq, VS]
            for v in range(NVT):
                lw_t = lmpool.tile([NP, NT, 512], F32R, name=f"lw_{v}", tag="lmw")
                nc.sync.dma_start(
                    lw_t[:], _r(d["lmw"][:, v * 512:(v + 1) * 512]))
                for tq in range(NT):
                    ps = psw.tile([NP, 512], F32, name=f"lmps_{v}_{tq}", tag="w")
                    for c in range(NT):
                        mm(ps[:], x_sb[:, c, tq * NP:(tq + 1) * NP],
                           lw_t[:, c, :], c == 0, c == NT - 1)
                    lo = apool.tile([NP, 512], F32, name=f"lo_{v}_{tq}", tag="lo")
                    if (v * NT + tq) % 2 == 0:
                        nc.scalar.copy(lo[:], ps[:])
                    else:
                        nc.vector.tensor_copy(lo[:], ps[:])
                    nc.sync.dma_start(
                        logits_d[tq * NP:(tq + 1) * NP, v * 512:(v + 1) * 512], lo[:])

    nc.compile()
    return nc


def _posenc():
    num_idx = (D + 1) // 2
    denom = (10000.0 ** (2.0 * np.arange(num_idx, dtype=np.float32) / D)).astype(np.float32)
    z = np.arange(TT, dtype=np.float32)[:, None] / denom[None]
    z_rep = np.repeat(z, 2, axis=1)[:, :D].astype(np.float32)
    idx = np.arange(D)
    return np.where(idx % 2 == 0, np.sin(z_rep), np.cos(z_rep)).astype(np.float32)


def _c(a):
    return np.ascontiguousarray(a, dtype=np.float32)


def prepare_in_maps(inputs):
    """Host-side sharding. Returns (in_maps, use_mask2, ln_affine, lm_b)."""
    gi = {k: np.asarray(v) for k, v in inputs.items()}
    target = gi["target"]
    emb = np.asarray(gi["emb"], np.float32)
    x0 = emb[target] + _posenc()[None]                      # [B, TT, D]
    tgt_pad = (target != V).astype(np.float32)              # [B, TT]
    tril = np.tril(np.ones((TT, TT), np.float32))
    mask1 = tgt_pad[:, None, :] * tgt_pad[:, :, None] * tril[None]    # [B, q, k]
    sp = gi["source_padding"].astype(np.float32)
    mask2 = sp[:, None, :] * tgt_pad[:, :, None]            # [B, q, k]
    use_mask2 = not np.all(mask2 == 1.0)
    ln_affine = not all(
        np.all(gi[f"ln{i}_g"] == 1.0) and np.all(gi[f"ln{i}_b"] == 0.0)
        for i in (1, 2, 3))

    # c vectors: sum_h bv[h] @ Wo[h*D:(h+1)*D] + bo  (all heads)
    cvec = {}
    for p, (wo, bv, bo) in {"1": (gi["Wo1"], gi["bv1"], gi["bo1"]),
                            "2": (gi["Wo2"], gi["bv2"], gi["bo2"])}.items():
        c = np.zeros((L, D), np.float32)
        for l in range(L):
            c[l] = bo[l] + np.einsum("hd,hde->e", bv[l],
                                     wo[l].reshape(H, D, D)).astype(np.float32)
        cvec[p] = c

    lmw_pad = np.zeros((D, 2 * VS), np.float32)
    lmw_pad[:, :V + 1] = gi["lm_W"]

    in_maps = []
    for core in range(8):
        b, hg = core // 2, core % 2
        hs = slice(hg * HH, (hg + 1) * HH)
        m = {
            "x0T": _c(x0[b].T),
            "esT": _c(gi["encoded_source"][b].T),
            "maskT": _c(mask1[b].T),
            "wff1": _c(gi["W_ff1"][:, :, hg * HID:(hg + 1) * HID]),
            "bff1": _c(gi["b_ff1"][:, hg * HID:(hg + 1) * HID]),
            "wff2": _c(gi["W_ff2"][:, hg * HID:(hg + 1) * HID, :]),
            "cff": _c(gi["b_ff2"]),
            "lmw": _c(lmw_pad[:, hg * VS:(hg + 1) * VS]),
        }
        if use_mask2:
            m["mask2T"] = _c(mask2[b].T)
        for p in ("1", "2"):
            m["wq" + p] = _c(gi["Wq" + p][:, hs])
            m["wk" + p] = _c(gi["Wk" + p][:, hs])
            m["wv" + p] = _c(gi["Wv" + p][:, hs])
            m["bq" + p] = _c(gi["bq" + p][:, hs])
            m["bk" + p] = _c(gi["bk" + p][:, hs])
            m["wo" + p] = _c(gi["Wo" + p][:, hg * HH * D:(hg + 1) * HH * D, :])
            m["c" + p] = _c(cvec[p])
        if ln_affine:
            m["lng"] = _c(np.stack([gi["ln1_g"], gi["ln2_g"], gi["ln3_g"]], axis=1))
            m["lnb"] = _c(np.stack([gi["ln1_b"], gi["ln2_b"], gi["ln3_b"]], axis=1))
        in_maps.append(m)
    return in_maps, use_mask2, ln_affine, np.asarray(gi["lm_b"], np.float32)


def assemble(results, lm_b):
    logits = np.zeros((B, TT, 2 * VS), np.float32)
    att0 = np.zeros((B, H, TT, TS), np.float32)
    for core in range(8):
        b, hg = core // 2, core % 2
        logits[b, :, hg * VS:(hg + 1) * VS] = results[core]["logits"]
        att0[b, hg * HH:(hg + 1) * HH] = np.transpose(
            results[core]["att0"], (0, 2, 1))
    out = logits[:, :, :V + 1] + lm_b[None, None, :]
    return out, att0


_NC_CACHE = {}


def kernel(**inputs):
    in_maps, use_mask2, ln_affine, lm_b = prepare_in_maps(inputs)
    key = (use_mask2, ln_affine)
    if key not in _NC_CACHE:
        _NC_CACHE[key] = build_nc(use_mask2, ln_affine)
    nc = _NC_CACHE[key]
    res = run_bass_kernel_spmd(nc, in_maps, list(range(8))).results
    return assemble(res, lm_b)


# revision 15
# speedup vs baseline: 1.0136x; 1.0136x over previous
"""Trainium2 Bass kernel for nn_Decoder_88605175316972.

Sharding: data-parallel over batch (4 samples) x tensor-parallel over heads
(2 groups of 4 heads) = 8 cores. Core c handles sample c//2, head-group c%2.
lm_W is column-sharded over vocab (each core computes 16384 padded columns).
Pairwise on-device AllReduce (replica groups [0,1],[2,3],[4,5],[6,7]) after
each attention output projection and each FFN second matmul. Logits and
layer-0 cross-attention weights are gathered on the host.

All activations are kept feature-major [feature(partitions), token(free)] so
layernorm/bias broadcasts are per-partition ops; softmax is computed on
transposed attention scores with row-sums done by ones-vector matmuls.
Matmuls run as float32r (full-rate fp32 on the PE).
"""

import numpy as np

import concourse.bass as bass
import concourse.mybir as mybir
import concourse.tile as tile
from concourse import bacc
from concourse.bass_utils import run_bass_kernel_spmd

V = 32000
D = 512
L = 2
H = 8
B = 4
TS = 512
TT = 512

NP = 128          # partitions
NT = D // NP      # 4 tiles per 512-dim
HH = H // 2       # heads per core
HID = 1024        # ffn hidden shard (4*D/2)
VS = 16384        # vocab shard per core (padded vocab 32768)
NVT = VS // 512   # vocab col-tiles per core
RG = [[0, 1], [2, 3], [4, 5], [6, 7]]
SCALE = 1.0 / float(np.sqrt(np.float32(D)))

F32 = mybir.dt.float32
F32R = mybir.dt.float32r
AF = mybir.ActivationFunctionType
ALU = mybir.AluOpType


def _r(ap, n):
    """host-packed [128, c*n] dram view -> [128, c, n]."""
    return ap.rearrange("p (c n) -> p c n", n=n)


def build_nc(use_mask2: bool, ln_affine: bool):
    nc = bacc.Bacc("TRN2", target_bir_lowering=False, debug=False, num_devices=8)
    d = {}

    def inp(name, shape, dt=F32R):
        d[name] = nc.dram_tensor(name, list(shape), dt, kind="ExternalInput")

    # all big tensors are host-packed partition-major: [..., 128, n*cols]
    inp("x0T", [NP, NT * TT])
    inp("esT", [NP, NT * TS])
    inp("maskT", [NP, NT * TT])
    if use_mask2:
        inp("mask2T", [NP, NT * TT])
    for p in ("1", "2"):
        inp("wq" + p, [L, HH, NP, NT * D])
        inp("wk" + p, [L, HH, NP, NT * D])
        inp("wv" + p, [L, HH, NP, NT * D])
        inp("bq" + p, [L, HH, D], F32)
        inp("bk" + p, [L, HH, D], F32)
        inp("wo" + p, [L, HH, NP, NT * D])
        inp("c" + p, [L, D], F32)
    inp("wff1", [L, NP, NT * HID])
    inp("bff1", [L, HID], F32)
    inp("wff2", [L, NP, (HID // NP) * D])
    inp("cff", [L, D], F32)
    if ln_affine:
        inp("lng", [L, 3, D], F32)
        inp("lnb", [L, 3, D], F32)
    inp("lmw", [NVT, NP, NT * 512])

    logits_d = nc.dram_tensor("logits", [TT, VS], F32, kind="ExternalOutput")
    att0_d = nc.dram_tensor("att0", [HH, TS, TT], F32, kind="ExternalOutput")

    def mm(ps, lhsT, rhs, start, stop):
        nc.tensor.matmul(ps, lhsT, rhs, start=start, stop=stop)

    with tile.TileContext(nc) as tc, \
         nc.allow_low_precision(reason="float32r matmul pipeline (tf32-class)"):
        with tc.tile_pool(name="const", bufs=1) as cpool, \
             tc.tile_pool(name="xp", bufs=2) as xpool, \
             tc.tile_pool(name="act", bufs=1) as apool, \
             tc.tile_pool(name="wp", bufs=1) as wpool, \
             tc.tile_pool(name="lmp", bufs=2) as lmpool, \
             tc.tile_pool(name="row", bufs=1) as rpool, \
             tc.tile_pool(name="psw", bufs=2, space="PSUM") as psw, \
             tc.tile_pool(name="psa", bufs=1, space="PSUM") as psa, \
             tc.tile_pool(name="psr", bufs=1, space="PSUM") as psr, \
             tc.tile_pool(name="dramp", bufs=2, space="DRAM") as dpool:

            ones_col = cpool.tile([NP, 1], F32R, name="ones_col")
            ones_row = cpool.tile([1, NP], F32R, name="ones_row")
            ones_f32 = cpool.tile([NP, 1], F32, name="ones_f32")
            eps_sb = cpool.tile([1, 1], F32, name="eps_sb")
            nc.vector.memset(ones_f32[:], 1.0)
            nc.vector.memset(eps_sb[:], 1e-5)
            nc.vector.tensor_copy(ones_col[:], ones_f32[:])
            nc.vector.tensor_copy(ones_row[:], ones_f32[:1, :1].to_broadcast([1, NP]))

            mask_sb = cpool.tile([NP, NT, TT], F32R, name="mask_sb")
            nc.sync.dma_start(mask_sb[:], _r(d["maskT"][:], TT))
            if use_mask2:
                mask2_sb = cpool.tile([NP, NT, TT], F32R, name="mask2_sb")
                nc.sync.dma_start(mask2_sb[:], _r(d["mask2T"][:], TT))
            es_sb = cpool.tile([NP, NT, TS], F32R, name="es_sb")
            nc.sync.dma_start(es_sb[:], _r(d["esT"][:], TS))

            # biases / small vectors, partition-tiled once
            bq_sb, bk_sb, c_sb = {}, {}, {}
            for p in ("1", "2"):
                bq_sb[p] = cpool.tile([NP, L, HH, NT], F32, name=f"bq{p}_sb")
                nc.sync.dma_start(
                    bq_sb[p][:], d["bq" + p][:].rearrange("l h (c p) -> p l h c", p=NP))
                bk_sb[p] = cpool.tile([NP, L, HH, NT], F32, name=f"bk{p}_sb")
                nc.sync.dma_start(
                    bk_sb[p][:], d["bk" + p][:].rearrange("l h (c p) -> p l h c", p=NP))
                c_sb[p] = cpool.tile([NP, L, NT], F32, name=f"c{p}_sb")
                nc.sync.dma_start(
                    c_sb[p][:], d["c" + p][:].rearrange("l (c p) -> p l c", p=NP))
            cff_sb = cpool.tile([NP, L, NT], F32, name="cff_sb")
            nc.sync.dma_start(cff_sb[:],
                              d["cff"][:].rearrange("l (c p) -> p l c", p=NP))
            bff1_sb = cpool.tile([NP, L, HID // NP], F32, name="bff1_sb")
            nc.sync.dma_start(bff1_sb[:],
                              d["bff1"][:].rearrange("l (c p) -> p l c", p=NP))
            if ln_affine:
                lng_sb = cpool.tile([NP, L, 3, NT], F32, name="lng_sb")
                nc.sync.dma_start(
                    lng_sb[:], d["lng"][:].rearrange("l k (c p) -> p l k c", p=NP))
                lnb_sb = cpool.tile([NP, L, 3, NT], F32, name="lnb_sb")
                nc.sync.dma_start(
                    lnb_sb[:], d["lnb"][:].rearrange("l k (c p) -> p l k c", p=NP))

            x_sb = xpool.tile([NP, NT, TT], F32R, name="x0", tag="x")
            nc.sync.dma_start(x_sb[:], _r(d["x0T"][:], TT))

            def all_reduce(src_sb, key):
                """src_sb [128, NT, 512] fm partial -> summed-over-pair tile."""
                ar_in = dpool.tile([NP, NT * TT], F32, name=f"arin_{key}", tag="ar_in")
                ar_out = dpool.tile([NP, NT * TT], F32, name=f"arout_{key}", tag="ar_out")
                nc.sync.dma_start(_r(ar_in, TT), src_sb[:])
                nc.gpsimd.collective_compute(
                    "AllReduce", ALU.add, replica_groups=RG,
                    ins=[ar_in.opt()], outs=[ar_out.opt()])
                af = xpool.tile([NP, NT, TT], F32, name=f"af_{key}", tag="af", bufs=1)
                nc.sync.dma_start(af[:], _r(ar_out, TT))
                return af

            def layernorm(resid, l, which, key):
                """resid [128, NT, 512] fm -> new x tile (normalized over feature dim)."""
                s1 = psr.tile([1, TT], F32, name=f"s1_{key}", tag="rs")
                sq = apool.tile([NP, NT, TT], F32R, name=f"sq_{key}", tag="sq")
                s2 = psr.tile([1, TT], F32, name=f"s2_{key}", tag="rs2")
                for c in range(NT):
                    mm(s1[:], ones_col[:], resid[:, c, :], c == 0, c == NT - 1)
                for c in range(NT):
                    nc.scalar.activation(sq[:, c, :], resid[:, c, :], AF.Square)
                for c in range(NT):
                    mm(s2[:], ones_col[:], sq[:, c, :], c == 0, c == NT - 1)
                mu = rpool.tile([1, TT], F32, name=f"mu_{key}", tag="mu")
                nc.vector.tensor_scalar_mul(mu[:], s1[:], 1.0 / D)
                # var = s2/D - mu^2, then sqrt(var+eps) in place
                var = rpool.tile([1, TT], F32, name=f"var_{key}", tag="var")
                nc.vector.tensor_mul(var[:], mu[:], mu[:])
                nc.vector.scalar_tensor_tensor(
                    var[:], s2[:], 1.0 / D, var[:], ALU.mult, ALU.subtract)
                nc.scalar.activation(var[:], var[:], AF.Sqrt, bias=eps_sb[:])
                rstd_f = rpool.tile([1, TT], F32, name=f"rstdf_{key}", tag="rinvf")
                scr = rpool.tile([1, TT], F32, name=f"lnscr_{key}", tag="scr")
                nc.vector.reciprocal_approx_accurate(rstd_f[:], var[:], scr[:])
                rstd = rpool.tile([1, TT], F32R, name=f"rstd_{key}", tag="rstd")
                nc.vector.tensor_copy(rstd[:], rstd_f[:])
                mr = rpool.tile([1, TT], F32R, name=f"mr_{key}", tag="mr")
                nc.vector.tensor_mul(mr[:], mu[:], rstd_f[:])
                rb_ps = psw.tile([NP, TT], F32, name=f"rbps_{key}", tag="w")
                mm(rb_ps[:], ones_row[:], rstd[:], True, True)
                rb = apool.tile([NP, TT], F32, name=f"rb_{key}", tag="rb")
                nc.scalar.copy(rb[:], rb_ps[:])
                mb_ps = psw.tile([NP, TT], F32, name=f"mbps_{key}", tag="w")
                mm(mb_ps[:], ones_row[:], mr[:], True, True)
                mb = apool.tile([NP, TT], F32, name=f"mb_{key}", tag="mb")
                nc.scalar.copy(mb[:], mb_ps[:])
                xn = xpool.tile([NP, NT, TT], F32R, name=f"x_{key}", tag="x")
                for c in range(NT):
                    nc.vector.tensor_mul(xn[:, c, :], resid[:, c, :], rb[:])
                    nc.vector.tensor_sub(xn[:, c, :], xn[:, c, :], mb[:])
                    if ln_affine:
                        nc.vector.tensor_scalar(
                            xn[:, c, :], xn[:, c, :], lng_sb[:, l, which, c:c + 1],
                            lnb_sb[:, l, which, c:c + 1], ALU.mult, ALU.add)
                return xn

            def residual_ln(x_old, af, cvec_ap, l, which, key):
                resid = apool.tile([NP, NT, TT], F32R, name=f"res_{key}", tag="res")
                for c in range(NT):
                    nc.vector.scalar_tensor_tensor(
                        resid[:, c, :], af[:, c, :], cvec_ap[c], x_old[:, c, :],
                        ALU.add, ALU.add)
                return resid

            def attention(x_cur, kv_sb, kv_len, p, l, msk, out_att0, key):
                """One masked MHA block (4 heads) -> all-reduced + LN'd new x."""
                a_ps = [psa.tile([NP, TT], F32, name=f"aps{dt}_{key}", tag=f"a{dt}")
                        for dt in range(NT)]
                NK = kv_len // NP
                for h in range(HH):
                    wq_t = wpool.tile([NP, NT, D], F32R, name=f"wq_{key}_{h}", tag="wq")
                    nc.sync.dma_start(wq_t[:], _r(d["wq" + p][l, h], D))
                    wk_t = wpool.tile([NP, NT, D], F32R, name=f"wk_{key}_{h}", tag="wk")
                    nc.sync.dma_start(wk_t[:], _r(d["wk" + p][l, h], D))
                    wv_t = wpool.tile([NP, NT, D], F32R, name=f"wv_{key}_{h}", tag="wv")
                    nc.sync.dma_start(wv_t[:], _r(d["wv" + p][l, h], D))
                    wo_t = wpool.tile([NP, NT, D], F32R, name=f"wo_{key}_{h}", tag="wo")
                    nc.sync.dma_start(wo_t[:], _r(d["wo" + p][l, h], D))

                    # Q/K feature-major [E, Tq] with fused per-partition bias
                    q_sb = apool.tile([NP, NT, TT], F32R, name=f"q_{key}_{h}", tag="q")
                    for e in range(NT):
                        ps = psw.tile([NP, TT], F32, name=f"qps_{key}_{h}_{e}", tag="w")
                        for c in range(NT):
                            mm(ps[:], wq_t[:, c, e * NP:(e + 1) * NP],
                               x_cur[:, c, :], c == 0, c == NT - 1)
                        nc.scalar.activation(q_sb[:, e, :], ps[:], AF.Identity,
                                             bias=bq_sb[p][:, l, h, e:e + 1])
                    k_sb = apool.tile([NP, NK, kv_len], F32R, name=f"k_{key}_{h}", tag="k")
                    for e in range(NT):
                        ps = psw.tile([NP, kv_len], F32, name=f"kps_{key}_{h}_{e}", tag="w")
                        for c in range(NT):
                            mm(ps[:], wk_t[:, c, e * NP:(e + 1) * NP],
                               kv_sb[:, c, :], c == 0, c == NT - 1)
                        nc.scalar.activation(k_sb[:, e, :], ps[:], AF.Identity,
                                             bias=bk_sb[p][:, l, h, e:e + 1])
                    # V token-major [Tk, E]
                    v_sb = apool.tile([NP, NK, D], F32R, name=f"v_{key}_{h}", tag="v")
                    for t in range(NK):
                        ps = psw.tile([NP, D], F32, name=f"vps_{key}_{h}_{t}", tag="w")
                        for c in range(NT):
                            mm(ps[:], kv_sb[:, c, t * NP:(t + 1) * NP],
                               wv_t[:, c, :], c == 0, c == NT - 1)
                        nc.scalar.copy(v_sb[:, t, :], ps[:])
                    # PT = exp(scale * K^T Q) * mask   [Tk, Tq]
                    pt_sb = apool.tile([NP, NK, TT], F32R, name=f"pt_{key}_{h}", tag="pt")
                    for t in range(NK):
                        ps = psw.tile([NP, TT], F32, name=f"ptps_{key}_{h}_{t}", tag="w")
                        for c in range(NT):
                            mm(ps[:], k_sb[:, c, t * NP:(t + 1) * NP],
                               q_sb[:, c, :], c == 0, c == NT - 1)
                        nc.scalar.activation(pt_sb[:, t, :], ps[:], AF.Exp, scale=SCALE)
                        if msk is not None:
                            nc.vector.tensor_mul(pt_sb[:, t, :], pt_sb[:, t, :],
                                                 msk[:, t, :])
                    # softmax denominator and reciprocal broadcast
                    rs = psr.tile([1, TT], F32, name=f"rs_{key}_{h}", tag="rs")
                    for t in range(NK):
                        mm(rs[:], ones_col[:], pt_sb[:, t, :], t == 0, t == NK - 1)
                    rinv_f = rpool.tile([1, TT], F32, name=f"rinvf_{key}_{h}", tag="rinvf")
                    scr = rpool.tile([1, TT], F32, name=f"scr_{key}_{h}", tag="scr")
                    nc.vector.reciprocal_approx_accurate(rinv_f[:], rs[:], scr[:])
                    rinv = rpool.tile([1, TT], F32R, name=f"rinv_{key}_{h}", tag="rinv")
                    nc.vector.tensor_copy(rinv[:], rinv_f[:])
                    bc_ps = psw.tile([NP, TT], F32, name=f"bcps_{key}_{h}", tag="w")
                    mm(bc_ps[:], ones_row[:], rinv[:], True, True)
                    bc = apool.tile([NP, TT], F32, name=f"bc_{key}_{h}", tag="bc")
                    nc.scalar.copy(bc[:], bc_ps[:])
                    # O^T = V^T P^T, normalized on evacuation  [E, Tq]
                    o_sb = apool.tile([NP, NT, TT], F32R, name=f"o_{key}_{h}", tag="o")
                    for e in range(NT):
                        ps = psw.tile([NP, TT], F32, name=f"ops_{key}_{h}_{e}", tag="w")
                        for t in range(NK):
                            mm(ps[:], v_sb[:, t, e * NP:(e + 1) * NP],
                               pt_sb[:, t, :], t == 0, t == NK - 1)
                        nc.vector.tensor_mul(o_sb[:, e, :], ps[:], bc[:])
                    if out_att0:
                        ptn = apool.tile([NP, NK, TT], F32, name=f"ptn_{key}_{h}", tag="sq")
                        for t in range(NK):
                            nc.vector.tensor_mul(ptn[:, t, :], pt_sb[:, t, :], bc[:])
                            nc.sync.dma_start(
                                att0_d[h, t * NP:(t + 1) * NP, :], ptn[:, t, :])
                    # accumulate Wo^T O^T into a_ps (feature-major [D, Tq])
                    for dt in range(NT):
                        for e in range(NT):
                            mm(a_ps[dt][:], wo_t[:, e, dt * NP:(dt + 1) * NP],
                               o_sb[:, e, :], h == 0 and e == 0,
                               h == HH - 1 and e == NT - 1)
                a_sb = apool.tile([NP, NT, TT], F32, name=f"a_{key}", tag="asb")
                for dt in range(NT):
                    nc.scalar.copy(a_sb[:, dt, :], a_ps[dt][:])
                af = all_reduce(a_sb, key)
                cvec = [c_sb[p][:, l, c:c + 1] for c in range(NT)]
                resid = residual_ln(x_cur, af, cvec, l, 0 if p == "1" else 1, key)
                return layernorm(resid, l, 0 if p == "1" else 1, key)

            for l in range(L):
                x_sb = attention(x_sb, x_sb, TT, "1", l, mask_sb, False, f"s{l}")
                x_sb = attention(x_sb, es_sb, TS, "2", l,
                                 mask2_sb if use_mask2 else None, l == 0, f"c{l}")
                # FFN: h1 = relu(W1^T x + b1) feature-major [HID, Tq]
                wf1_t = wpool.tile([NP, NT, HID], F32R, name=f"wf1_{l}", tag="wff")
                nc.sync.dma_start(wf1_t[:], _r(d["wff1"][l], HID))
                h1_sb = apool.tile([NP, HID // NP, TT], F32R, name=f"h1_{l}", tag="h1")
                for m in range(HID // NP):
                    ps = psw.tile([NP, TT], F32, name=f"h1ps_{l}_{m}", tag="w")
                    for c in range(NT):
                        mm(ps[:], wf1_t[:, c, m * NP:(m + 1) * NP],
                           x_sb[:, c, :], c == 0, c == NT - 1)
                    nc.scalar.activation(h1_sb[:, m, :], ps[:], AF.Relu,
                                         bias=bff1_sb[:, l, m:m + 1])
                wf2_t = wpool.tile([NP, HID // NP, D], F32R, name=f"wf2_{l}", tag="wff")
                nc.sync.dma_start(wf2_t[:], _r(d["wff2"][l], D))
                f_ps = [psa.tile([NP, TT], F32, name=f"fps{dt}_{l}", tag=f"a{dt}")
                        for dt in range(NT)]
                for dt in range(NT):
                    for c in range(HID // NP):
                        mm(f_ps[dt][:], wf2_t[:, c, dt * NP:(dt + 1) * NP],
                           h1_sb[:, c, :], c == 0, c == HID // NP - 1)
                ff_sb = apool.tile([NP, NT, TT], F32, name=f"ff_{l}", tag="asb")
                for dt in range(NT):
                    nc.scalar.copy(ff_sb[:, dt, :], f_ps[dt][:])
                af = all_reduce(ff_sb, f"f{l}")
                cvec = [cff_sb[:, l, c:c + 1] for c in range(NT)]
                resid = residual_ln(x_sb, af, cvec, l, 2, f"f{l}")
                x_sb = layernorm(resid, l, 2, f"f{l}")

            # LM head: logits token-major [Tq, VS]
            for v in range(NVT):
                lw_t = lmpool.tile([NP, NT, 512], F32R, name=f"lw_{v}", tag="lmw")
                nc.sync.dma_start(lw_t[:], _r(d["lmw"][v], 512))
                for tq in range(NT):
                    i = v * NT + tq
                    if i % 3 == 0:
                        ps = psw.tile([NP, 512], F32, name=f"lmps_{v}_{tq}", tag="w")
                    else:
                        ps = psa.tile([NP, 512], F32, name=f"lmps_{v}_{tq}",
                                      tag=f"a{i % 3}")
                    for c in range(NT):
                        mm(ps[:], x_sb[:, c, tq * NP:(tq + 1) * NP],
                           lw_t[:, c, :], c == 0, c == NT - 1)
                    lo = apool.tile([NP, 512], F32, name=f"lo_{v}_{tq}", tag="lo")
                    if (v * NT + tq) % 2 == 0:
                        nc.scalar.copy(lo[:], ps[:])
                    else:
                        nc.vector.tensor_copy(lo[:], ps[:])
                    nc.sync.dma_start(
                        logits_d[tq * NP:(tq + 1) * NP, v * 512:(v + 1) * 512], lo[:])

    nc.compile()
    return nc


def _posenc():
    num_idx = (D + 1) // 2
    denom = (10000.0 ** (2.0 * np.arange(num_idx, dtype=np.float32) / D)).astype(np.float32)
    z = np.arange(TT, dtype=np.float32)[:, None] / denom[None]
    z_rep = np.repeat(z, 2, axis=1)[:, :D].astype(np.float32)
    idx = np.arange(D)
    return np.where(idx % 2 == 0, np.sin(z_rep), np.cos(z_rep)).astype(np.float32)


def _c(a):
    return np.ascontiguousarray(a, dtype=np.float32)


def _pm(a):
    """[..., n*128, C] -> [..., 128, n*C] partition-major packing."""
    *lead, R, C = a.shape
    n = R // NP
    a = a.reshape(*lead, n, NP, C)
    a = np.moveaxis(a, -3, -2)
    return np.ascontiguousarray(a.reshape(*lead, NP, n * C), dtype=np.float32)


def prepare_in_maps(inputs):
    """Host-side sharding. Returns (in_maps, use_mask2, ln_affine, lm_b)."""
    gi = {k: np.asarray(v) for k, v in inputs.items()}
    target = gi["target"]
    emb = np.asarray(gi["emb"], np.float32)
    x0 = emb[target] + _posenc()[None]                      # [B, TT, D]
    tgt_pad = (target != V).astype(np.float32)              # [B, TT]
    tril = np.tril(np.ones((TT, TT), np.float32))
    mask1 = tgt_pad[:, None, :] * tgt_pad[:, :, None] * tril[None]    # [B, q, k]
    sp = gi["source_padding"].astype(np.float32)
    mask2 = sp[:, None, :] * tgt_pad[:, :, None]            # [B, q, k]
    use_mask2 = not np.all(mask2 == 1.0)
    ln_affine = not all(
        np.all(gi[f"ln{i}_g"] == 1.0) and np.all(gi[f"ln{i}_b"] == 0.0)
        for i in (1, 2, 3))

    # c vectors: sum_h bv[h] @ Wo[h*D:(h+1)*D] + bo  (all heads)
    cvec = {}
    for p, (wo, bv, bo) in {"1": (gi["Wo1"], gi["bv1"], gi["bo1"]),
                            "2": (gi["Wo2"], gi["bv2"], gi["bo2"])}.items():
        c = np.zeros((L, D), np.float32)
        for l in range(L):
            c[l] = bo[l] + np.einsum("hd,hde->e", bv[l],
                                     wo[l].reshape(H, D, D)).astype(np.float32)
        cvec[p] = c

    lmw_pad = np.zeros((D, 2 * VS), np.float32)
    lmw_pad[:, :V + 1] = gi["lm_W"]

    in_maps = []
    for core in range(8):
        b, hg = core // 2, core % 2
        hs = slice(hg * HH, (hg + 1) * HH)
        lmw_s = lmw_pad[:, hg * VS:(hg + 1) * VS]           # [D, VS]
        lmw_vt = np.stack([_pm(lmw_s[:, v * 512:(v + 1) * 512])
                           for v in range(NVT)])             # [NVT, 128, NT*512]
        m = {
            "x0T": _pm(x0[b].T),
            "esT": _pm(gi["encoded_source"][b].T),
            "maskT": _pm(mask1[b].T),
            "wff1": _pm(gi["W_ff1"][:, :, hg * HID:(hg + 1) * HID]),
            "bff1": _c(gi["b_ff1"][:, hg * HID:(hg + 1) * HID]),
            "wff2": _pm(gi["W_ff2"][:, hg * HID:(hg + 1) * HID, :]),
            "cff": _c(gi["b_ff2"]),
            "lmw": lmw_vt,
        }
        if use_mask2:
            m["mask2T"] = _pm(mask2[b].T)
        for p in ("1", "2"):
            m["wq" + p] = _pm(gi["Wq" + p][:, hs])
            m["wk" + p] = _pm(gi["Wk" + p][:, hs])
            m["wv" + p] = _pm(gi["Wv" + p][:, hs])
            m["bq" + p] = _c(gi["bq" + p][:, hs])
            m["bk" + p] = _c(gi["bk" + p][:, hs])
            m["wo" + p] = _pm(gi["Wo" + p][:, hg * HH * D:(hg + 1) * HH * D, :]
                              .reshape(L, HH, D, D))
            m["c" + p] = _c(cvec[p])
        if ln_affine:
            m["lng"] = _c(np.stack([gi["ln1_g"], gi["ln2_g"], gi["ln3_g"]], axis=1))
            m["lnb"] = _c(np.stack([gi["ln1_b"], gi["ln2_b"], gi["ln3_b"]], axis=1))
        in_maps.append(m)
    return in_maps, use_mask2, ln_affine, np.asarray(gi["lm_b"], np.float32)


def assemble(results, lm_b):
    logits = np.zeros((B, TT, 2 * VS), np.float32)
    att0 = np.zeros((B, H, TT, TS), np.float32)
    for core in range(8):
        b, hg = core // 2, core % 2
        logits[b, :, hg * VS:(hg + 1) * VS] = results[core]["logits"]
        att0[b, hg * HH:(hg + 1) * HH] = np.transpose(
            results[core]["att0"], (0, 2, 1))
    out = logits[:, :, :V + 1] + lm_b[None, None, :]
    return out, att0


_NC_CACHE = {}


def kernel(**inputs):
    in_maps, use_mask2, ln_affine, lm_b = prepare_in_maps(inputs)
    key = (use_mask2, ln_affine)
    if key not in _NC_CACHE:
        _NC_CACHE[key] = build_nc(use_mask2, ln_affine)
    nc = _NC_CACHE[key]
    res = run_bass_kernel_spmd(nc, in_maps, list(range(8))).results
    return assemble(res, lm_b)


# revision 18
# speedup vs baseline: 1.0524x; 1.0383x over previous
"""Trainium2 Bass kernel for nn_Decoder_88605175316972.

Sharding: data-parallel over batch (4 samples) x tensor-parallel over heads
(2 groups of 4 heads) = 8 cores. Core c handles sample c//2, head-group c%2.
lm_W is column-sharded over vocab (each core computes 16384 padded columns).
Pairwise on-device AllReduce (replica groups [0,1],[2,3],[4,5],[6,7]) after
each attention output projection and each FFN second matmul. Logits and
layer-0 cross-attention weights are gathered on the host.

All activations are kept feature-major [feature(partitions), token(free)] so
layernorm/bias broadcasts are per-partition ops; softmax is computed on
transposed attention scores with row-sums done by ones-vector matmuls.
Matmuls run as float32r (full-rate fp32 on the PE).
"""

import numpy as np

import concourse.bass as bass
import concourse.mybir as mybir
import concourse.tile as tile
from concourse import bacc
from concourse.bass_utils import run_bass_kernel_spmd

V = 32000
D = 512
L = 2
H = 8
B = 4
TS = 512
TT = 512

NP = 128          # partitions
NT = D // NP      # 4 tiles per 512-dim
HH = H // 2       # heads per core
HID = 1024        # ffn hidden shard (4*D/2)
VS = 16384        # vocab shard per core (padded vocab 32768)
NVT = VS // 512   # vocab col-tiles per core
RG = [[0, 1], [2, 3], [4, 5], [6, 7]]
SCALE = 1.0 / float(np.sqrt(np.float32(D)))

F32 = mybir.dt.float32
F32R = mybir.dt.float32r
AF = mybir.ActivationFunctionType
ALU = mybir.AluOpType


def _r(ap, n):
    """host-packed [128, c*n] dram view -> [128, c, n]."""
    return ap.rearrange("p (c n) -> p c n", n=n)


def build_nc(use_mask2: bool, ln_affine: bool):
    nc = bacc.Bacc("TRN2", target_bir_lowering=False, debug=False, num_devices=8)
    d = {}

    def inp(name, shape, dt=F32R):
        d[name] = nc.dram_tensor(name, list(shape), dt, kind="ExternalInput")

    # all big tensors are host-packed partition-major: [..., 128, n*cols]
    inp("x0T", [NP, NT * TT])
    inp("esT", [NP, NT * TS])
    inp("maskT", [NP, NT * TT])
    if use_mask2:
        inp("mask2T", [NP, NT * TT])
    for p in ("1", "2"):
        inp("wq" + p, [L, HH, NP, NT * D])
        inp("wk" + p, [L, HH, NP, NT * D])
        inp("wv" + p, [L, HH, NP, NT * D])
        inp("bq" + p, [L, HH, D], F32)
        inp("bk" + p, [L, HH, D], F32)
        inp("wo" + p, [L, HH, NP, NT * D])
        inp("c" + p, [L, D], F32)
    inp("wff1", [L, NP, NT * HID])
    inp("bff1", [L, HID], F32)
    inp("wff2", [L, NP, (HID // NP) * D])
    inp("cff", [L, D], F32)
    if ln_affine:
        inp("lng", [L, 3, D], F32)
        inp("lnb", [L, 3, D], F32)
    inp("lmw", [NVT, NP, NT * 512])

    logits_d = nc.dram_tensor("logits", [TT, VS], F32, kind="ExternalOutput")
    att0_d = nc.dram_tensor("att0", [HH, TS, TT], F32, kind="ExternalOutput")

    def mm(ps, lhsT, rhs, start, stop):
        nc.tensor.matmul(ps, lhsT, rhs, start=start, stop=stop)

    with tile.TileContext(nc) as tc, \
         nc.allow_low_precision(reason="float32r matmul pipeline (tf32-class)"):
        with tc.tile_pool(name="const", bufs=1) as cpool, \
             tc.tile_pool(name="xp", bufs=2) as xpool, \
             tc.tile_pool(name="act", bufs=1) as apool, \
             tc.tile_pool(name="wp", bufs=1) as wpool, \
             tc.tile_pool(name="lmp", bufs=2) as lmpool, \
             tc.tile_pool(name="row", bufs=1) as rpool, \
             tc.tile_pool(name="psw", bufs=2, space="PSUM") as psw, \
             tc.tile_pool(name="psa", bufs=1, space="PSUM") as psa, \
             tc.tile_pool(name="psr", bufs=1, space="PSUM") as psr, \
             tc.tile_pool(name="dramp", bufs=2, space="DRAM") as dpool:

            ones_col = cpool.tile([NP, 1], F32R, name="ones_col")
            ones_row = cpool.tile([1, NP], F32R, name="ones_row")
            ones_f32 = cpool.tile([NP, 1], F32, name="ones_f32")
            eps_sb = cpool.tile([1, 1], F32, name="eps_sb")
            nc.vector.memset(ones_f32[:], 1.0)
            nc.vector.memset(eps_sb[:], 1e-5)
            nc.vector.tensor_copy(ones_col[:], ones_f32[:])
            nc.vector.tensor_copy(ones_row[:], ones_f32[:1, :1].to_broadcast([1, NP]))

            mask_sb = cpool.tile([NP, NT, TT], F32R, name="mask_sb")
            nc.sync.dma_start(mask_sb[:], _r(d["maskT"][:], TT))
            if use_mask2:
                mask2_sb = cpool.tile([NP, NT, TT], F32R, name="mask2_sb")
                nc.sync.dma_start(mask2_sb[:], _r(d["mask2T"][:], TT))
            es_sb = cpool.tile([NP, NT, TS], F32R, name="es_sb")
            nc.sync.dma_start(es_sb[:], _r(d["esT"][:], TS))

            # biases / small vectors, partition-tiled once
            bq_sb, bk_sb, c_sb = {}, {}, {}
            for p in ("1", "2"):
                bq_sb[p] = cpool.tile([NP, L, HH, NT], F32, name=f"bq{p}_sb")
                nc.sync.dma_start(
                    bq_sb[p][:], d["bq" + p][:].rearrange("l h (c p) -> p l h c", p=NP))
                bk_sb[p] = cpool.tile([NP, L, HH, NT], F32, name=f"bk{p}_sb")
                nc.sync.dma_start(
                    bk_sb[p][:], d["bk" + p][:].rearrange("l h (c p) -> p l h c", p=NP))
                c_sb[p] = cpool.tile([NP, L, NT], F32, name=f"c{p}_sb")
                nc.sync.dma_start(
                    c_sb[p][:], d["c" + p][:].rearrange("l (c p) -> p l c", p=NP))
            cff_sb = cpool.tile([NP, L, NT], F32, name="cff_sb")
            nc.sync.dma_start(cff_sb[:],
                              d["cff"][:].rearrange("l (c p) -> p l c", p=NP))
            bff1_sb = cpool.tile([NP, L, HID // NP], F32, name="bff1_sb")
            nc.sync.dma_start(bff1_sb[:],
                              d["bff1"][:].rearrange("l (c p) -> p l c", p=NP))
            if ln_affine:
                lng_sb = cpool.tile([NP, L, 3, NT], F32, name="lng_sb")
                nc.sync.dma_start(
                    lng_sb[:], d["lng"][:].rearrange("l k (c p) -> p l k c", p=NP))
                lnb_sb = cpool.tile([NP, L, 3, NT], F32, name="lnb_sb")
                nc.sync.dma_start(
                    lnb_sb[:], d["lnb"][:].rearrange("l k (c p) -> p l k c", p=NP))

            x_sb = xpool.tile([NP, NT, TT], F32R, name="x0", tag="x")
            nc.sync.dma_start(x_sb[:], _r(d["x0T"][:], TT))

            def all_reduce(src_sb, key):
                """src_sb [128, NT, 512] fm partial -> summed-over-pair tile."""
                ar_in = dpool.tile([NP, NT * TT], F32, name=f"arin_{key}", tag="ar_in")
                ar_out = dpool.tile([NP, NT * TT], F32, name=f"arout_{key}", tag="ar_out")
                nc.sync.dma_start(_r(ar_in, TT), src_sb[:])
                nc.gpsimd.collective_compute(
                    "AllReduce", ALU.add, replica_groups=RG,
                    ins=[ar_in.opt()], outs=[ar_out.opt()])
                af = apool.tile([NP, NT, TT], F32, name=f"af_{key}", tag="asb")
                nc.sync.dma_start(af[:], _r(ar_out, TT))
                return af

            def layernorm(resid, l, which, key):
                """resid [128, NT, 512] fm -> new x tile (normalized over feature dim)."""
                s1 = psr.tile([1, TT], F32, name=f"s1_{key}", tag="rs")
                sq = apool.tile([NP, NT, TT], F32R, name=f"sq_{key}", tag="sq")
                s2 = psr.tile([1, TT], F32, name=f"s2_{key}", tag="rs2")
                for c in range(NT):
                    mm(s1[:], ones_col[:], resid[:, c, :], c == 0, c == NT - 1)
                for c in range(NT):
                    nc.scalar.activation(sq[:, c, :], resid[:, c, :], AF.Square)
                for c in range(NT):
                    mm(s2[:], ones_col[:], sq[:, c, :], c == 0, c == NT - 1)
                mu = rpool.tile([1, TT], F32, name=f"mu_{key}", tag="mu")
                nc.vector.tensor_scalar_mul(mu[:], s1[:], 1.0 / D)
                # var = s2/D - mu^2, then sqrt(var+eps) in place
                var = rpool.tile([1, TT], F32, name=f"var_{key}", tag="var")
                nc.vector.tensor_mul(var[:], mu[:], mu[:])
                nc.vector.scalar_tensor_tensor(
                    var[:], s2[:], 1.0 / D, var[:], ALU.mult, ALU.subtract)
                nc.scalar.activation(var[:], var[:], AF.Sqrt, bias=eps_sb[:])
                rstd_f = rpool.tile([1, TT], F32, name=f"rstdf_{key}", tag="rinvf")
                scr = rpool.tile([1, TT], F32, name=f"lnscr_{key}", tag="scr")
                nc.vector.reciprocal_approx_accurate(rstd_f[:], var[:], scr[:])
                rstd = rpool.tile([1, TT], F32R, name=f"rstd_{key}", tag="rstd")
                nc.vector.tensor_copy(rstd[:], rstd_f[:])
                mr = rpool.tile([1, TT], F32R, name=f"mr_{key}", tag="mr")
                nc.vector.tensor_mul(mr[:], mu[:], rstd_f[:])
                rb_ps = psw.tile([NP, TT], F32, name=f"rbps_{key}", tag="w")
                mm(rb_ps[:], ones_row[:], rstd[:], True, True)
                rb = apool.tile([NP, TT], F32, name=f"rb_{key}", tag="rb")
                nc.scalar.copy(rb[:], rb_ps[:])
                mb_ps = psw.tile([NP, TT], F32, name=f"mbps_{key}", tag="w")
                mm(mb_ps[:], ones_row[:], mr[:], True, True)
                mb = apool.tile([NP, TT], F32, name=f"mb_{key}", tag="mb")
                nc.scalar.copy(mb[:], mb_ps[:])
                xn = xpool.tile([NP, NT, TT], F32R, name=f"x_{key}", tag="x")
                for c in range(NT):
                    nc.vector.tensor_mul(xn[:, c, :], resid[:, c, :], rb[:])
                    nc.vector.tensor_sub(xn[:, c, :], xn[:, c, :], mb[:])
                    if ln_affine:
                        nc.vector.tensor_scalar(
                            xn[:, c, :], xn[:, c, :], lng_sb[:, l, which, c:c + 1],
                            lnb_sb[:, l, which, c:c + 1], ALU.mult, ALU.add)
                return xn

            def residual_ln(x_old, af, cvec_ap, l, which, key):
                resid = apool.tile([NP, NT, TT], F32R, name=f"res_{key}", tag="res")
                for c in range(NT):
                    nc.vector.scalar_tensor_tensor(
                        resid[:, c, :], af[:, c, :], cvec_ap[c], x_old[:, c, :],
                        ALU.add, ALU.add)
                return resid

            def make_kv(kv_sb, kv_len, p, l, h, key):
                """K feature-major + V token-major for one head (x-independent
                for cross-attention -> can be emitted into AllReduce gaps)."""
                NK = kv_len // NP
                wk_t = wpool.tile([NP, NT, D], F32R, name=f"wk_{key}_{h}", tag="wk")
                nc.sync.dma_start(wk_t[:], _r(d["wk" + p][l, h], D))
                wv_t = wpool.tile([NP, NT, D], F32R, name=f"wv_{key}_{h}", tag="wv")
                nc.sync.dma_start(wv_t[:], _r(d["wv" + p][l, h], D))
                k_sb = apool.tile([NP, NK, kv_len], F32R, name=f"k_{key}_{h}", tag="k")
                for e in range(NT):
                    ps = psw.tile([NP, kv_len], F32, name=f"kps_{key}_{h}_{e}", tag="w")
                    for c in range(NT):
                        mm(ps[:], wk_t[:, c, e * NP:(e + 1) * NP],
                           kv_sb[:, c, :], c == 0, c == NT - 1)
                    nc.scalar.activation(k_sb[:, e, :], ps[:], AF.Identity,
                                         bias=bk_sb[p][:, l, h, e:e + 1])
                v_sb = apool.tile([NP, NK, D], F32R, name=f"v_{key}_{h}", tag="v")
                for t in range(NK):
                    ps = psw.tile([NP, D], F32, name=f"vps_{key}_{h}_{t}", tag="w")
                    for c in range(NT):
                        mm(ps[:], kv_sb[:, c, t * NP:(t + 1) * NP],
                           wv_t[:, c, :], c == 0, c == NT - 1)
                    nc.scalar.copy(v_sb[:, t, :], ps[:])
                return k_sb, v_sb

            def attention(x_cur, kv_sb, kv_len, p, l, msk, out_att0, key,
                          kv_pre=None, pre_emit=None):
                """One masked MHA block (4 heads) -> all-reduced + LN'd new x."""
                a_ps = [psa.tile([NP, TT], F32, name=f"aps{dt}_{key}", tag=f"a{dt}")
                        for dt in range(NT)]
                NK = kv_len // NP
                for h in range(HH):
                    wq_t = wpool.tile([NP, NT, D], F32R, name=f"wq_{key}_{h}", tag="wq")
                    nc.sync.dma_start(wq_t[:], _r(d["wq" + p][l, h], D))
                    wo_t = wpool.tile([NP, NT, D], F32R, name=f"wo_{key}_{h}", tag="wo")
                    nc.sync.dma_start(wo_t[:], _r(d["wo" + p][l, h], D))

                    # Q/K feature-major [E, Tq] with fused per-partition bias
                    q_sb = apool.tile([NP, NT, TT], F32R, name=f"q_{key}_{h}", tag="q", bufs=2)
                    for e in range(NT):
                        ps = psw.tile([NP, TT], F32, name=f"qps_{key}_{h}_{e}", tag="w")
                        for c in range(NT):
                            mm(ps[:], wq_t[:, c, e * NP:(e + 1) * NP],
                               x_cur[:, c, :], c == 0, c == NT - 1)
                        nc.scalar.activation(q_sb[:, e, :], ps[:], AF.Identity,
                                             bias=bq_sb[p][:, l, h, e:e + 1])
                    if h == 0 and kv_pre is not None:
                        k_sb, v_sb = kv_pre
                    else:
                        k_sb, v_sb = make_kv(kv_sb, kv_len, p, l, h, key)
                    # PT = exp(scale * K^T Q) * mask   [Tk, Tq]
                    pt_sb = apool.tile([NP, NK, TT], F32R, name=f"pt_{key}_{h}", tag="pt")
                    for t in range(NK):
                        ps = psw.tile([NP, TT], F32, name=f"ptps_{key}_{h}_{t}", tag="w")
                        for c in range(NT):
                            mm(ps[:], k_sb[:, c, t * NP:(t + 1) * NP],
                               q_sb[:, c, :], c == 0, c == NT - 1)
                        nc.scalar.activation(pt_sb[:, t, :], ps[:], AF.Exp, scale=SCALE)
                        if msk is not None:
                            nc.vector.tensor_mul(pt_sb[:, t, :], pt_sb[:, t, :],
                                                 msk[:, t, :])
                    # softmax denominator and reciprocal broadcast
                    rs = psr.tile([1, TT], F32, name=f"rs_{key}_{h}", tag="rs")
                    for t in range(NK):
                        mm(rs[:], ones_col[:], pt_sb[:, t, :], t == 0, t == NK - 1)
                    rinv_f = rpool.tile([1, TT], F32, name=f"rinvf_{key}_{h}", tag="rinvf")
                    scr = rpool.tile([1, TT], F32, name=f"scr_{key}_{h}", tag="scr")
                    nc.vector.reciprocal_approx_accurate(rinv_f[:], rs[:], scr[:])
                    rinv = rpool.tile([1, TT], F32R, name=f"rinv_{key}_{h}", tag="rinv")
                    nc.vector.tensor_copy(rinv[:], rinv_f[:])
                    bc_ps = psw.tile([NP, TT], F32, name=f"bcps_{key}_{h}", tag="w")
                    mm(bc_ps[:], ones_row[:], rinv[:], True, True)
                    bc = apool.tile([NP, TT], F32, name=f"bc_{key}_{h}", tag="bc")
                    nc.scalar.copy(bc[:], bc_ps[:])
                    # O^T = V^T P^T, normalized on evacuation  [E, Tq]
                    o_sb = apool.tile([NP, NT, TT], F32R, name=f"o_{key}_{h}", tag="o")
                    for e in range(NT):
                        ps = psw.tile([NP, TT], F32, name=f"ops_{key}_{h}_{e}", tag="w")
                        for t in range(NK):
                            mm(ps[:], v_sb[:, t, e * NP:(e + 1) * NP],
                               pt_sb[:, t, :], t == 0, t == NK - 1)
                        nc.vector.tensor_mul(o_sb[:, e, :], ps[:], bc[:])
                    if out_att0:
                        ptn = apool.tile([NP, NK, TT], F32, name=f"ptn_{key}_{h}", tag="sq")
                        for t in range(NK):
                            nc.vector.tensor_mul(ptn[:, t, :], pt_sb[:, t, :], bc[:])
                            nc.sync.dma_start(
                                att0_d[h, t * NP:(t + 1) * NP, :], ptn[:, t, :])
                    # accumulate Wo^T O^T into a_ps (feature-major [D, Tq])
                    for dt in range(NT):
                        for e in range(NT):
                            mm(a_ps[dt][:], wo_t[:, e, dt * NP:(dt + 1) * NP],
                               o_sb[:, e, :], h == 0 and e == 0,
                               h == HH - 1 and e == NT - 1)
                a_sb = apool.tile([NP, NT, TT], F32, name=f"a_{key}", tag="asb")
                for dt in range(NT):
                    nc.scalar.copy(a_sb[:, dt, :], a_ps[dt][:])
                kv_next = None
                if pre_emit is not None:
                    pe_kv_sb, pe_kv_len, pe_p, pe_l, pe_key = pre_emit
                    kv_next = make_kv(pe_kv_sb, pe_kv_len, pe_p, pe_l, 0, pe_key)
                af = all_reduce(a_sb, key)
                cvec = [c_sb[p][:, l, c:c + 1] for c in range(NT)]
                resid = residual_ln(x_cur, af, cvec, l, 0 if p == "1" else 1, key)
                return layernorm(resid, l, 0 if p == "1" else 1, key), kv_next

            for l in range(L):
                x_sb, ckv = attention(
                    x_sb, x_sb, TT, "1", l, mask_sb, False, f"s{l}",
                    pre_emit=(es_sb, TS, "2", l, f"c{l}"))
                x_sb, _ = attention(
                    x_sb, es_sb, TS, "2", l,
                    mask2_sb if use_mask2 else None, l == 0, f"c{l}",
                    kv_pre=ckv)
                # FFN: h1 = relu(W1^T x + b1) feature-major [HID, Tq]
                wf1_t = wpool.tile([NP, NT, HID], F32R, name=f"wf1_{l}", tag="wff")
                nc.sync.dma_start(wf1_t[:], _r(d["wff1"][l], HID))
                h1_sb = apool.tile([NP, HID // NP, TT], F32R, name=f"h1_{l}", tag="h1")
                for m in range(HID // NP):
                    ps = psw.tile([NP, TT], F32, name=f"h1ps_{l}_{m}", tag="w")
                    for c in range(NT):
                        mm(ps[:], wf1_t[:, c, m * NP:(m + 1) * NP],
                           x_sb[:, c, :], c == 0, c == NT - 1)
                    nc.scalar.activation(h1_sb[:, m, :], ps[:], AF.Relu,
                                         bias=bff1_sb[:, l, m:m + 1])
                wf2_t = wpool.tile([NP, HID // NP, D], F32R, name=f"wf2_{l}", tag="wff")
                nc.sync.dma_start(wf2_t[:], _r(d["wff2"][l], D))
                f_ps = [psa.tile([NP, TT], F32, name=f"fps{dt}_{l}", tag=f"a{dt}")
                        for dt in range(NT)]
                for dt in range(NT):
                    for c in range(HID // NP):
                        mm(f_ps[dt][:], wf2_t[:, c, dt * NP:(dt + 1) * NP],
                           h1_sb[:, c, :], c == 0, c == HID // NP - 1)
                ff_sb = apool.tile([NP, NT, TT], F32, name=f"ff_{l}", tag="asb")
                for dt in range(NT):
                    nc.scalar.copy(ff_sb[:, dt, :], f_ps[dt][:])
                af = all_reduce(ff_sb, f"f{l}")
                cvec = [cff_sb[:, l, c:c + 1] for c in range(NT)]
                resid = residual_ln(x_sb, af, cvec, l, 2, f"f{l}")
                x_sb = layernorm(resid, l, 2, f"f{l}")

            # LM head: logits token-major [Tq, VS]
            for v in range(NVT):
                lw_t = lmpool.tile([NP, NT, 512], F32R, name=f"lw_{v}", tag="lmw")
                nc.sync.dma_start(lw_t[:], _r(d["lmw"][v], 512))
                for tq in range(NT):
                    i = v * NT + tq
                    if i % 3 == 0:
                        ps = psw.tile([NP, 512], F32, name=f"lmps_{v}_{tq}", tag="w")
                    else:
                        ps = psa.tile([NP, 512], F32, name=f"lmps_{v}_{tq}",
                                      tag=f"a{i % 3}")
                    for c in range(NT):
                        mm(ps[:], x_sb[:, c, tq * NP:(tq + 1) * NP],
                           lw_t[:, c, :], c == 0, c == NT - 1)
                    lo = apool.tile([NP, 512], F32, name=f"lo_{v}_{tq}", tag="lo")
                    if (v * NT + tq) % 2 == 0:
                        nc.scalar.copy(lo[:], ps[:])
                    else:
                        nc.vector.tensor_copy(lo[:], ps[:])
                    nc.sync.dma_start(
                        logits_d[tq * NP:(tq + 1) * NP, v * 512:(v + 1) * 512], lo[:])

    nc.compile()
    return nc


def _posenc():
    num_idx = (D + 1) // 2
    denom = (10000.0 ** (2.0 * np.arange(num_idx, dtype=np.float32) / D)).astype(np.float32)
    z = np.arange(TT, dtype=np.float32)[:, None] / denom[None]
    z_rep = np.repeat(z, 2, axis=1)[:, :D].astype(np.float32)
    idx = np.arange(D)
    return np.where(idx % 2 == 0, np.sin(z_rep), np.cos(z_rep)).astype(np.float32)


def _c(a):
    return np.ascontiguousarray(a, dtype=np.float32)


def _pm(a):
    """[..., n*128, C] -> [..., 128, n*C] partition-major packing."""
    *lead, R, C = a.shape
    n = R // NP
    a = a.reshape(*lead, n, NP, C)
    a = np.moveaxis(a, -3, -2)
    return np.ascontiguousarray(a.reshape(*lead, NP, n * C), dtype=np.float32)


def prepare_in_maps(inputs):
    """Host-side sharding. Returns (in_maps, use_mask2, ln_affine, lm_b)."""
    gi = {k: np.asarray(v) for k, v in inputs.items()}
    target = gi["target"]
    emb = np.asarray(gi["emb"], np.float32)
    x0 = emb[target] + _posenc()[None]                      # [B, TT, D]
    tgt_pad = (target != V).astype(np.float32)              # [B, TT]
    tril = np.tril(np.ones((TT, TT), np.float32))
    mask1 = tgt_pad[:, None, :] * tgt_pad[:, :, None] * tril[None]    # [B, q, k]
    sp = gi["source_padding"].astype(np.float32)
    mask2 = sp[:, None, :] * tgt_pad[:, :, None]            # [B, q, k]
    use_mask2 = not np.all(mask2 == 1.0)
    ln_affine = not all(
        np.all(gi[f"ln{i}_g"] == 1.0) and np.all(gi[f"ln{i}_b"] == 0.0)
        for i in (1, 2, 3))

    # c vectors: sum_h bv[h] @ Wo[h*D:(h+1)*D] + bo  (all heads)
    cvec = {}
    for p, (wo, bv, bo) in {"1": (gi["Wo1"], gi["bv1"], gi["bo1"]),
                            "2": (gi["Wo2"], gi["bv2"], gi["bo2"])}.items():
        c = np.zeros((L, D), np.float32)
        for l in range(L):
            c[l] = bo[l] + np.einsum("hd,hde->e", bv[l],
                                     wo[l].reshape(H, D, D)).astype(np.float32)
        cvec[p] = c

    lmw_pad = np.zeros((D, 2 * VS), np.float32)
    lmw_pad[:, :V + 1] = gi["lm_W"]

    in_maps = []
    for core in range(8):
        b, hg = core // 2, core % 2
        hs = slice(hg * HH, (hg + 1) * HH)
        lmw_s = lmw_pad[:, hg * VS:(hg + 1) * VS]           # [D, VS]
        lmw_vt = np.stack([_pm(lmw_s[:, v * 512:(v + 1) * 512])
                           for v in range(NVT)])             # [NVT, 128, NT*512]
        m = {
            "x0T": _pm(x0[b].T),
            "esT": _pm(gi["encoded_source"][b].T),
            "maskT": _pm(mask1[b].T),
            "wff1": _pm(gi["W_ff1"][:, :, hg * HID:(hg + 1) * HID]),
            "bff1": _c(gi["b_ff1"][:, hg * HID:(hg + 1) * HID]),
            "wff2": _pm(gi["W_ff2"][:, hg * HID:(hg + 1) * HID, :]),
            "cff": _c(gi["b_ff2"]),
            "lmw": lmw_vt,
        }
        if use_mask2:
            m["mask2T"] = _pm(mask2[b].T)
        for p in ("1", "2"):
            m["wq" + p] = _pm(gi["Wq" + p][:, hs])
            m["wk" + p] = _pm(gi["Wk" + p][:, hs])
            m["wv" + p] = _pm(gi["Wv" + p][:, hs])
            m["bq" + p] = _c(gi["bq" + p][:, hs])
            m["bk" + p] = _c(gi["bk" + p][:, hs])
            m["wo" + p] = _pm(gi["Wo" + p][:, hg * HH * D:(hg + 1) * HH * D, :]
                              .reshape(L, HH, D, D))
            m["c" + p] = _c(cvec[p])
        if ln_affine:
            m["lng"] = _c(np.stack([gi["ln1_g"], gi["ln2_g"], gi["ln3_g"]], axis=1))
            m["lnb"] = _c(np.stack([gi["ln1_b"], gi["ln2_b"], gi["ln3_b"]], axis=1))
        in_maps.append(m)
    return in_maps, use_mask2, ln_affine, np.asarray(gi["lm_b"], np.float32)


def assemble(results, lm_b):
    logits = np.zeros((B, TT, 2 * VS), np.float32)
    att0 = np.zeros((B, H, TT, TS), np.float32)
    for core in range(8):
        b, hg = core // 2, core % 2
        logits[b, :, hg * VS:(hg + 1) * VS] = results[core]["logits"]
        att0[b, hg * HH:(hg + 1) * HH] = np.transpose(
            results[core]["att0"], (0, 2, 1))
    out = logits[:, :, :V + 1] + lm_b[None, None, :]
    return out, att0


_NC_CACHE = {}


def kernel(**inputs):
    in_maps, use_mask2, ln_affine, lm_b = prepare_in_maps(inputs)
    key = (use_mask2, ln_affine)
    if key not in _NC_CACHE:
        _NC_CACHE[key] = build_nc(use_mask2, ln_affine)
    nc = _NC_CACHE[key]
    res = run_bass_kernel_spmd(nc, in_maps, list(range(8))).results
    return assemble(res, lm_b)


# revision 23
# speedup vs baseline: 1.0839x; 1.0299x over previous
"""Trainium2 Bass kernel for nn_Decoder_88605175316972.

Sharding: data-parallel over batch (4 samples) x tensor-parallel over heads
(2 groups of 4 heads) = 8 cores. Core c handles sample c//2, head-group c%2.
lm_W is column-sharded over vocab (each core computes 16384 padded columns).
Pairwise on-device AllReduce (replica groups [0,1],[2,3],[4,5],[6,7]) after
each attention output projection and each FFN second matmul. Logits and
layer-0 cross-attention weights are gathered on the host.

All activations are kept feature-major [feature(partitions), token(free)] so
layernorm/bias broadcasts are per-partition ops; softmax is computed on
transposed attention scores with row-sums done by ones-vector matmuls.
Matmuls run as float32r (full-rate fp32 on the PE).
"""

import numpy as np

import concourse.bass as bass
import concourse.mybir as mybir
import concourse.tile as tile
from concourse import bacc
from concourse.bass_utils import run_bass_kernel_spmd

V = 32000
D = 512
L = 2
H = 8
B = 4
TS = 512
TT = 512

NP = 128          # partitions
NT = D // NP      # 4 tiles per 512-dim
HH = H // 2       # heads per core
HID = 1024        # ffn hidden shard (4*D/2)
VS = 16384        # vocab shard per core (padded vocab 32768)
NVT = VS // 512   # vocab col-tiles per core
RG = [[0, 1], [2, 3], [4, 5], [6, 7]]
SCALE = 1.0 / float(np.sqrt(np.float32(D)))

F32 = mybir.dt.float32
F32R = mybir.dt.float32r
AF = mybir.ActivationFunctionType
ALU = mybir.AluOpType


def _r(ap, n):
    """host-packed [128, c*n] dram view -> [128, c, n]."""
    return ap.rearrange("p (c n) -> p c n", n=n)


def build_nc(use_mask2: bool, ln_affine: bool):
    nc = bacc.Bacc("TRN2", target_bir_lowering=False, debug=False, num_devices=8)
    d = {}

    def inp(name, shape, dt=F32R):
        d[name] = nc.dram_tensor(name, list(shape), dt, kind="ExternalInput")

    # all big tensors are host-packed partition-major: [..., 128, n*cols]
    inp("x0T", [NP, NT * TT])
    inp("esT", [NP, NT * TS])
    inp("maskT", [NP, NT * TT])
    if use_mask2:
        inp("mask2T", [NP, NT * TT])
    for p in ("1", "2"):
        inp("wq" + p, [L, HH, NP, NT * D])
        inp("wk" + p, [L, HH, NP, NT * D])
        inp("wv" + p, [L, HH, NP, NT * D])
        inp("bq" + p, [L, HH, D], F32)
        inp("bk" + p, [L, HH, D], F32)
        inp("wo" + p, [L, HH, NP, NT * D])
        inp("c" + p, [L, D], F32)
    inp("wff1", [L, NP, NT * HID])
    inp("bff1", [L, HID], F32)
    inp("wff2", [L, NP, (HID // NP) * D])
    inp("cff", [L, D], F32)
    if ln_affine:
        inp("lng", [L, 3, D], F32)
        inp("lnb", [L, 3, D], F32)
    inp("lmw", [NVT, NP, NT * 512])

    logits_d = nc.dram_tensor("logits", [TT, VS], F32, kind="ExternalOutput")
    att0_d = nc.dram_tensor("att0", [HH, TS, TT], F32, kind="ExternalOutput")

    def mm(ps, lhsT, rhs, start, stop):
        nc.tensor.matmul(ps, lhsT, rhs, start=start, stop=stop)

    with tile.TileContext(nc) as tc, \
         nc.allow_low_precision(reason="float32r matmul pipeline (tf32-class)"):
        with tc.tile_pool(name="const", bufs=1) as cpool, \
             tc.tile_pool(name="xp", bufs=2) as xpool, \
             tc.tile_pool(name="act", bufs=1) as apool, \
             tc.tile_pool(name="wp", bufs=1) as wpool, \
             tc.tile_pool(name="lmp", bufs=2) as lmpool, \
             tc.tile_pool(name="row", bufs=1) as rpool, \
             tc.tile_pool(name="psw", bufs=2, space="PSUM") as psw, \
             tc.tile_pool(name="psa", bufs=1, space="PSUM") as psa, \
             tc.tile_pool(name="psr", bufs=1, space="PSUM") as psr, \
             tc.tile_pool(name="dramp", bufs=2, space="DRAM") as dpool:

            ones_col = cpool.tile([NP, 1], F32R, name="ones_col")
            ones_row = cpool.tile([1, NP], F32R, name="ones_row")
            ones_f32 = cpool.tile([NP, 1], F32, name="ones_f32")
            eps_sb = cpool.tile([1, 1], F32, name="eps_sb")
            nc.vector.memset(ones_f32[:], 1.0)
            nc.vector.memset(eps_sb[:], 1e-5)
            nc.vector.tensor_copy(ones_col[:], ones_f32[:])
            nc.vector.tensor_copy(ones_row[:], ones_f32[:1, :1].to_broadcast([1, NP]))

            mask_sb = cpool.tile([NP, NT, TT], F32R, name="mask_sb")
            nc.sync.dma_start(mask_sb[:], _r(d["maskT"][:], TT))
            if use_mask2:
                mask2_sb = cpool.tile([NP, NT, TT], F32R, name="mask2_sb")
                nc.sync.dma_start(mask2_sb[:], _r(d["mask2T"][:], TT))
            es_sb = cpool.tile([NP, NT, TS], F32R, name="es_sb")
            nc.sync.dma_start(es_sb[:], _r(d["esT"][:], TS))

            # biases / small vectors, partition-tiled once
            bq_sb, bk_sb, c_sb = {}, {}, {}
            for p in ("1", "2"):
                bq_sb[p] = cpool.tile([NP, L, HH, NT], F32, name=f"bq{p}_sb")
                nc.sync.dma_start(
                    bq_sb[p][:], d["bq" + p][:].rearrange("l h (c p) -> p l h c", p=NP))
                bk_sb[p] = cpool.tile([NP, L, HH, NT], F32, name=f"bk{p}_sb")
                nc.sync.dma_start(
                    bk_sb[p][:], d["bk" + p][:].rearrange("l h (c p) -> p l h c", p=NP))
                c_sb[p] = cpool.tile([NP, L, NT], F32, name=f"c{p}_sb")
                nc.sync.dma_start(
                    c_sb[p][:], d["c" + p][:].rearrange("l (c p) -> p l c", p=NP))
            cff_sb = cpool.tile([NP, L, NT], F32, name="cff_sb")
            nc.sync.dma_start(cff_sb[:],
                              d["cff"][:].rearrange("l (c p) -> p l c", p=NP))
            bff1_sb = cpool.tile([NP, L, HID // NP], F32, name="bff1_sb")
            nc.sync.dma_start(bff1_sb[:],
                              d["bff1"][:].rearrange("l (c p) -> p l c", p=NP))
            if ln_affine:
                lng_sb = cpool.tile([NP, L, 3, NT], F32, name="lng_sb")
                nc.sync.dma_start(
                    lng_sb[:], d["lng"][:].rearrange("l k (c p) -> p l k c", p=NP))
                lnb_sb = cpool.tile([NP, L, 3, NT], F32, name="lnb_sb")
                nc.sync.dma_start(
                    lnb_sb[:], d["lnb"][:].rearrange("l k (c p) -> p l k c", p=NP))

            x_sb = xpool.tile([NP, NT, TT], F32R, name="x0", tag="x")
            nc.sync.dma_start(x_sb[:], _r(d["x0T"][:], TT))

            def all_reduce(src_sb, key):
                """src_sb [128, NT, 512] fm partial -> summed-over-pair tile."""
                ar_in = dpool.tile([NP, NT * TT], F32R, name=f"arin_{key}", tag="ar_in")
                ar_out = dpool.tile([NP, NT * TT], F32R, name=f"arout_{key}", tag="ar_out")
                nc.sync.dma_start(_r(ar_in, TT), src_sb[:])
                nc.gpsimd.collective_compute(
                    "AllReduce", ALU.add, replica_groups=RG,
                    ins=[ar_in.opt()], outs=[ar_out.opt()])
                af = apool.tile([NP, NT, TT], F32R, name=f"af_{key}", tag="asb")
                nc.sync.dma_start(af[:], _r(ar_out, TT))
                return af

            def layernorm(resid, l, which, key):
                """resid [128, NT, 512] fm -> new x tile (normalized over feature dim)."""
                s1 = psr.tile([1, TT], F32, name=f"s1_{key}", tag="rs")
                sq = xpool.tile([NP, NT, TT], F32R, name=f"sq_{key}", tag="x")
                s2 = psr.tile([1, TT], F32, name=f"s2_{key}", tag="rs2")
                for c in range(NT):
                    mm(s1[:], ones_col[:], resid[:, c, :], c == 0, c == NT - 1)
                for c in range(NT):
                    nc.scalar.activation(sq[:, c, :], resid[:, c, :], AF.Square)
                for c in range(NT):
                    mm(s2[:], ones_col[:], sq[:, c, :], c == 0, c == NT - 1)
                mu = apool.tile([1, TT], F32, name=f"mu_{key}", tag="mb")
                nc.vector.tensor_scalar_mul(mu[:], s1[:], 1.0 / D)
                # var = s2/D - mu^2, then sqrt(var+eps) in place
                var = rpool.tile([1, TT], F32, name=f"var_{key}", tag="var")
                nc.vector.tensor_mul(var[:], mu[:], mu[:])
                nc.vector.scalar_tensor_tensor(
                    var[:], s2[:], 1.0 / D, var[:], ALU.mult, ALU.subtract)
                nc.scalar.activation(var[:], var[:], AF.Sqrt, bias=eps_sb[:])
                rstd_f = rpool.tile([1, TT], F32, name=f"rstdf_{key}", tag="rinvf")
                scr = rpool.tile([1, TT], F32, name=f"lnscr_{key}", tag="scr")
                nc.vector.reciprocal_approx_accurate(rstd_f[:], var[:], scr[:])
                rstd = rpool.tile([1, TT], F32R, name=f"rstd_{key}", tag="rinv")
                nc.vector.tensor_copy(rstd[:], rstd_f[:])
                mr = rpool.tile([1, TT], F32R, name=f"mr_{key}", tag="mr")
                nc.vector.tensor_mul(mr[:], mu[:], rstd_f[:])
                rb_ps = psw.tile([NP, TT], F32, name=f"rbps_{key}", tag="w")
                mm(rb_ps[:], ones_row[:], rstd[:], True, True)
                rb = apool.tile([NP, TT], F32, name=f"rb_{key}", tag="bc")
                nc.scalar.copy(rb[:], rb_ps[:])
                mb_ps = psw.tile([NP, TT], F32, name=f"mbps_{key}", tag="w")
                mm(mb_ps[:], ones_row[:], mr[:], True, True)
                mb = apool.tile([NP, TT], F32, name=f"mb_{key}", tag="mb")
                nc.scalar.copy(mb[:], mb_ps[:])
                xn = xpool.tile([NP, NT, TT], F32R, name=f"x_{key}", tag="x")
                for c in range(NT):
                    nc.vector.tensor_mul(xn[:, c, :], resid[:, c, :], rb[:])
                    nc.vector.tensor_sub(xn[:, c, :], xn[:, c, :], mb[:])
                    if ln_affine:
                        nc.vector.tensor_scalar(
                            xn[:, c, :], xn[:, c, :], lng_sb[:, l, which, c:c + 1],
                            lnb_sb[:, l, which, c:c + 1], ALU.mult, ALU.add)
                return xn

            def residual_ln(x_old, af, cvec_ap, l, which, key):
                resid = apool.tile([NP, NT, TT], F32R, name=f"res_{key}", tag="res")
                for c in range(NT):
                    nc.vector.scalar_tensor_tensor(
                        resid[:, c, :], af[:, c, :], cvec_ap[c], x_old[:, c, :],
                        ALU.add, ALU.add)
                return resid

            def make_kv(kv_sb, kv_len, p, l, h, key):
                """K feature-major + V token-major for one head (x-independent
                for cross-attention -> can be emitted into AllReduce gaps)."""
                NK = kv_len // NP
                wk_t = wpool.tile([NP, NT, D], F32R, name=f"wk_{key}_{h}", tag="wk", bufs=2)
                nc.sync.dma_start(wk_t[:], _r(d["wk" + p][l, h], D))
                wv_t = wpool.tile([NP, NT, D], F32R, name=f"wv_{key}_{h}", tag="wv")
                nc.sync.dma_start(wv_t[:], _r(d["wv" + p][l, h], D))
                k_sb = apool.tile([NP, NK, kv_len], F32R, name=f"k_{key}_{h}", tag="k")
                for e in range(NT):
                    ps = psw.tile([NP, kv_len], F32, name=f"kps_{key}_{h}_{e}", tag="w")
                    for c in range(NT):
                        mm(ps[:], wk_t[:, c, e * NP:(e + 1) * NP],
                           kv_sb[:, c, :], c == 0, c == NT - 1)
                    nc.scalar.activation(k_sb[:, e, :], ps[:], AF.Identity,
                                         bias=bk_sb[p][:, l, h, e:e + 1])
                v_sb = apool.tile([NP, NK, D], F32R, name=f"v_{key}_{h}", tag="v")
                for t in range(NK):
                    ps = psw.tile([NP, D], F32, name=f"vps_{key}_{h}_{t}", tag="w")
                    for c in range(NT):
                        mm(ps[:], kv_sb[:, c, t * NP:(t + 1) * NP],
                           wv_t[:, c, :], c == 0, c == NT - 1)
                    nc.scalar.copy(v_sb[:, t, :], ps[:])
                return k_sb, v_sb

            def attention(x_cur, kv_sb, kv_len, p, l, msk, out_att0, key,
                          kv_pre=None, pre_emit=None):
                """One masked MHA block (4 heads) -> all-reduced + LN'd new x."""
                a_ps = [psa.tile([NP, TT], F32, name=f"aps{dt}_{key}", tag=f"a{dt}")
                        for dt in range(NT)]
                NK = kv_len // NP
                for h in range(HH):
                    wq_t = wpool.tile([NP, NT, D], F32R, name=f"wq_{key}_{h}", tag="wq")
                    nc.sync.dma_start(wq_t[:], _r(d["wq" + p][l, h], D))
                    wo_t = wpool.tile([NP, NT, D], F32R, name=f"wo_{key}_{h}", tag="wo")
                    nc.sync.dma_start(wo_t[:], _r(d["wo" + p][l, h], D))

                    # Q/K feature-major [E, Tq] with fused per-partition bias
                    q_sb = apool.tile([NP, NT, TT], F32R, name=f"q_{key}_{h}", tag="q", bufs=2)
                    for e in range(NT):
                        ps = psw.tile([NP, TT], F32, name=f"qps_{key}_{h}_{e}", tag="w")
                        for c in range(NT):
                            mm(ps[:], wq_t[:, c, e * NP:(e + 1) * NP],
                               x_cur[:, c, :], c == 0, c == NT - 1)
                        nc.scalar.activation(q_sb[:, e, :], ps[:], AF.Identity,
                                             bias=bq_sb[p][:, l, h, e:e + 1])
                    if h == 0 and kv_pre is not None:
                        k_sb, v_sb = kv_pre
                    else:
                        k_sb, v_sb = make_kv(kv_sb, kv_len, p, l, h, key)
                    # PT = exp(scale * K^T Q) * mask   [Tk, Tq]
                    pt_sb = apool.tile([NP, NK, TT], F32R, name=f"pt_{key}_{h}", tag="pt")
                    for t in range(NK):
                        ps = psw.tile([NP, TT], F32, name=f"ptps_{key}_{h}_{t}", tag="w")
                        for c in range(NT):
                            mm(ps[:], k_sb[:, c, t * NP:(t + 1) * NP],
                               q_sb[:, c, :], c == 0, c == NT - 1)
                        nc.scalar.activation(pt_sb[:, t, :], ps[:], AF.Exp, scale=SCALE)
                        if msk is not None:
                            nc.vector.tensor_mul(pt_sb[:, t, :], pt_sb[:, t, :],
                                                 msk[:, t, :])
                    # softmax denominator and reciprocal broadcast
                    rs = psr.tile([1, TT], F32, name=f"rs_{key}_{h}", tag="rs")
                    for t in range(NK):
                        mm(rs[:], ones_col[:], pt_sb[:, t, :], t == 0, t == NK - 1)
                    rinv_f = rpool.tile([1, TT], F32, name=f"rinvf_{key}_{h}", tag="rinvf")
                    scr = rpool.tile([1, TT], F32, name=f"scr_{key}_{h}", tag="scr")
                    nc.vector.reciprocal_approx_accurate(rinv_f[:], rs[:], scr[:])
                    rinv = rpool.tile([1, TT], F32R, name=f"rinv_{key}_{h}", tag="rinv")
                    nc.vector.tensor_copy(rinv[:], rinv_f[:])
                    bc_ps = psw.tile([NP, TT], F32, name=f"bcps_{key}_{h}", tag="w")
                    mm(bc_ps[:], ones_row[:], rinv[:], True, True)
                    bc = apool.tile([NP, TT], F32, name=f"bc_{key}_{h}", tag="bc")
                    nc.scalar.copy(bc[:], bc_ps[:])
                    # O^T = V^T P^T, normalized on evacuation  [E, Tq]
                    o_sb = apool.tile([NP, NT, TT], F32R, name=f"o_{key}_{h}", tag="o")
                    for e in range(NT):
                        if e % 3 == 2:
                            ps = psr.tile([NP, TT], F32, name=f"ops_{key}_{h}_{e}",
                                          tag="rs2")
                        else:
                            ps = psw.tile([NP, TT], F32, name=f"ops_{key}_{h}_{e}",
                                          tag="w")
                        for t in range(NK):
                            mm(ps[:], v_sb[:, t, e * NP:(e + 1) * NP],
                               pt_sb[:, t, :], t == 0, t == NK - 1)
                        nc.vector.tensor_mul(o_sb[:, e, :], ps[:], bc[:])
                    if out_att0:
                        ptn = apool.tile([NP, NK, TT], F32, name=f"ptn_{key}_{h}", tag="sq")
                        for t in range(NK):
                            nc.vector.tensor_mul(ptn[:, t, :], pt_sb[:, t, :], bc[:])
                            nc.sync.dma_start(
                                att0_d[h, t * NP:(t + 1) * NP, :], ptn[:, t, :])
                    # accumulate Wo^T O^T into a_ps (feature-major [D, Tq])
                    for dt in range(NT):
                        for e in range(NT):
                            mm(a_ps[dt][:], wo_t[:, e, dt * NP:(dt + 1) * NP],
                               o_sb[:, e, :], h == 0 and e == 0,
                               h == HH - 1 and e == NT - 1)
                a_sb = apool.tile([NP, NT, TT], F32R, name=f"a_{key}", tag="asb")
                for dt in range(NT):
                    nc.scalar.copy(a_sb[:, dt, :], a_ps[dt][:])
                kv_next = None
                if pre_emit is not None:
                    pe_kv_sb, pe_kv_len, pe_p, pe_l, pe_key = pre_emit
                    kv_next = make_kv(pe_kv_sb, pe_kv_len, pe_p, pe_l, 0, pe_key)
                af = all_reduce(a_sb, key)
                cvec = [c_sb[p][:, l, c:c + 1] for c in range(NT)]
                resid = residual_ln(x_cur, af, cvec, l, 0 if p == "1" else 1, key)
                return layernorm(resid, l, 0 if p == "1" else 1, key), kv_next

            for l in range(L):
                x_sb, ckv = attention(
                    x_sb, x_sb, TT, "1", l, mask_sb, False, f"s{l}",
                    pre_emit=(es_sb, TS, "2", l, f"c{l}"))
                x_sb, _ = attention(
                    x_sb, es_sb, TS, "2", l,
                    mask2_sb if use_mask2 else None, l == 0, f"c{l}",
                    kv_pre=ckv)
                # FFN: h1 = relu(W1^T x + b1) feature-major [HID, Tq]
                wf1_t = wpool.tile([NP, NT, HID], F32R, name=f"wf1_{l}", tag="wff")
                nc.sync.dma_start(wf1_t[:], _r(d["wff1"][l], HID))
                h1_sb = apool.tile([NP, HID // NP, TT], F32R, name=f"h1_{l}", tag="h1")
                for m in range(HID // NP):
                    ps = psw.tile([NP, TT], F32, name=f"h1ps_{l}_{m}", tag="w")
                    for c in range(NT):
                        mm(ps[:], wf1_t[:, c, m * NP:(m + 1) * NP],
                           x_sb[:, c, :], c == 0, c == NT - 1)
                    nc.scalar.activation(h1_sb[:, m, :], ps[:], AF.Relu,
                                         bias=bff1_sb[:, l, m:m + 1])
                wf2_t = wpool.tile([NP, HID // NP, D], F32R, name=f"wf2_{l}", tag="wff")
                nc.sync.dma_start(wf2_t[:], _r(d["wff2"][l], D))
                f_ps = [psa.tile([NP, TT], F32, name=f"fps{dt}_{l}", tag=f"a{dt}")
                        for dt in range(NT)]
                for dt in range(NT):
                    for c in range(HID // NP):
                        mm(f_ps[dt][:], wf2_t[:, c, dt * NP:(dt + 1) * NP],
                           h1_sb[:, c, :], c == 0, c == HID // NP - 1)
                ff_sb = apool.tile([NP, NT, TT], F32R, name=f"ff_{l}", tag="asb")
                for dt in range(NT):
                    nc.scalar.copy(ff_sb[:, dt, :], f_ps[dt][:])
                af = all_reduce(ff_sb, f"f{l}")
                cvec = [cff_sb[:, l, c:c + 1] for c in range(NT)]
                resid = residual_ln(x_sb, af, cvec, l, 2, f"f{l}")
                x_sb = layernorm(resid, l, 2, f"f{l}")

            # LM head: logits token-major [Tq, VS]
            for v in range(NVT):
                lw_t = lmpool.tile([NP, NT, 512], F32R, name=f"lw_{v}", tag="lmw")
                nc.sync.dma_start(lw_t[:], _r(d["lmw"][v], 512))
                for tq in range(NT):
                    i = v * NT + tq
                    if i % 3 == 0:
                        ps = psw.tile([NP, 512], F32, name=f"lmps_{v}_{tq}", tag="w")
                    else:
                        ps = psa.tile([NP, 512], F32, name=f"lmps_{v}_{tq}",
                                      tag=f"a{i % 3}")
                    for c in range(NT):
                        mm(ps[:], x_sb[:, c, tq * NP:(tq + 1) * NP],
                           lw_t[:, c, :], c == 0, c == NT - 1)
                    lo = apool.tile([NP, 512], F32, name=f"lo_{v}_{tq}", tag="bc")
                    if (v * NT + tq) % 2 == 0:
                        nc.scalar.copy(lo[:], ps[:])
                    else:
                        nc.vector.tensor_copy(lo[:], ps[:])
                    nc.sync.dma_start(
                        logits_d[tq * NP:(tq + 1) * NP, v * 512:(v + 1) * 512], lo[:])

    nc.compile()
    return nc


def _posenc():
    num_idx = (D + 1) // 2
    denom = (10000.0 ** (2.0 * np.arange(num_idx, dtype=np.float32) / D)).astype(np.float32)
    z = np.arange(TT, dtype=np.float32)[:, None] / denom[None]
    z_rep = np.repeat(z, 2, axis=1)[:, :D].astype(np.float32)
    idx = np.arange(D)
    return np.where(idx % 2 == 0, np.sin(z_rep), np.cos(z_rep)).astype(np.float32)


def _c(a):
    return np.ascontiguousarray(a, dtype=np.float32)


def _pm(a):
    """[..., n*128, C] -> [..., 128, n*C] partition-major packing."""
    *lead, R, C = a.shape
    n = R // NP
    a = a.reshape(*lead, n, NP, C)
    a = np.moveaxis(a, -3, -2)
    return np.ascontiguousarray(a.reshape(*lead, NP, n * C), dtype=np.float32)


def prepare_in_maps(inputs):
    """Host-side sharding. Returns (in_maps, use_mask2, ln_affine, lm_b)."""
    gi = {k: np.asarray(v) for k, v in inputs.items()}
    target = gi["target"]
    emb = np.asarray(gi["emb"], np.float32)
    x0 = emb[target] + _posenc()[None]                      # [B, TT, D]
    tgt_pad = (target != V).astype(np.float32)              # [B, TT]
    tril = np.tril(np.ones((TT, TT), np.float32))
    mask1 = tgt_pad[:, None, :] * tgt_pad[:, :, None] * tril[None]    # [B, q, k]
    sp = gi["source_padding"].astype(np.float32)
    mask2 = sp[:, None, :] * tgt_pad[:, :, None]            # [B, q, k]
    use_mask2 = not np.all(mask2 == 1.0)
    ln_affine = not all(
        np.all(gi[f"ln{i}_g"] == 1.0) and np.all(gi[f"ln{i}_b"] == 0.0)
        for i in (1, 2, 3))

    # c vectors: sum_h bv[h] @ Wo[h*D:(h+1)*D] + bo  (all heads)
    cvec = {}
    for p, (wo, bv, bo) in {"1": (gi["Wo1"], gi["bv1"], gi["bo1"]),
                            "2": (gi["Wo2"], gi["bv2"], gi["bo2"])}.items():
        c = np.zeros((L, D), np.float32)
        for l in range(L):
            c[l] = bo[l] + np.einsum("hd,hde->e", bv[l],
                                     wo[l].reshape(H, D, D)).astype(np.float32)
        cvec[p] = c

    lmw_pad = np.zeros((D, 2 * VS), np.float32)
    lmw_pad[:, :V + 1] = gi["lm_W"]

    in_maps = []
    for core in range(8):
        b, hg = core // 2, core % 2
        hs = slice(hg * HH, (hg + 1) * HH)
        lmw_s = lmw_pad[:, hg * VS:(hg + 1) * VS]           # [D, VS]
        lmw_vt = np.stack([_pm(lmw_s[:, v * 512:(v + 1) * 512])
                           for v in range(NVT)])             # [NVT, 128, NT*512]
        m = {
            "x0T": _pm(x0[b].T),
            "esT": _pm(gi["encoded_source"][b].T),
            "maskT": _pm(mask1[b].T),
            "wff1": _pm(gi["W_ff1"][:, :, hg * HID:(hg + 1) * HID]),
            "bff1": _c(gi["b_ff1"][:, hg * HID:(hg + 1) * HID]),
            "wff2": _pm(gi["W_ff2"][:, hg * HID:(hg + 1) * HID, :]),
            "cff": _c(gi["b_ff2"]),
            "lmw": lmw_vt,
        }
        if use_mask2:
            m["mask2T"] = _pm(mask2[b].T)
        for p in ("1", "2"):
            m["wq" + p] = _pm(gi["Wq" + p][:, hs])
            m["wk" + p] = _pm(gi["Wk" + p][:, hs])
            m["wv" + p] = _pm(gi["Wv" + p][:, hs])
            m["bq" + p] = _c(gi["bq" + p][:, hs])
            m["bk" + p] = _c(gi["bk" + p][:, hs])
            m["wo" + p] = _pm(gi["Wo" + p][:, hg * HH * D:(hg + 1) * HH * D, :]
                              .reshape(L, HH, D, D))
            m["c" + p] = _c(cvec[p])
        if ln_affine:
            m["lng"] = _c(np.stack([gi["ln1_g"], gi["ln2_g"], gi["ln3_g"]], axis=1))
            m["lnb"] = _c(np.stack([gi["ln1_b"], gi["ln2_b"], gi["ln3_b"]], axis=1))
        in_maps.append(m)
    return in_maps, use_mask2, ln_affine, np.asarray(gi["lm_b"], np.float32)


def assemble(results, lm_b):
    logits = np.zeros((B, TT, 2 * VS), np.float32)
    att0 = np.zeros((B, H, TT, TS), np.float32)
    for core in range(8):
        b, hg = core // 2, core % 2
        logits[b, :, hg * VS:(hg + 1) * VS] = results[core]["logits"]
        att0[b, hg * HH:(hg + 1) * HH] = np.transpose(
            results[core]["att0"], (0, 2, 1))
    out = logits[:, :, :V + 1] + lm_b[None, None, :]
    return out, att0


_NC_CACHE = {}


def kernel(**inputs):
    in_maps, use_mask2, ln_affine, lm_b = prepare_in_maps(inputs)
    key = (use_mask2, ln_affine)
    if key not in _NC_CACHE:
        _NC_CACHE[key] = build_nc(use_mask2, ln_affine)
    nc = _NC_CACHE[key]
    res = run_bass_kernel_spmd(nc, in_maps, list(range(8))).results
    return assemble(res, lm_b)
